# revision 11
# baseline (speedup 1.0000x reference)
"""Causal self-attention on 8 Trainium2 NeuronCores.

Problem (hardcoded): B=4, T=2048, C=1024, H=16, D=64.
  qkv = x @ w_qkv + b_qkv ; per-head causal softmax attention ; out = attn @ w_proj + b_proj

Sharding (per hint): tensor-parallel over heads x data-parallel over batch.
  core c -> batch b = c // 2, head group g = c % 2 (heads g*8 .. g*8+7).
Each core computes QKV for its 8 heads, causal attention, and a partial
projection (its 512 input channels of w_proj). Host sums the two partials per
batch and adds b_proj.

On-core layout ("transposed" attention so softmax reduction lands on the
matmul contraction axis):
  xT   [C, T]  (host pre-transposed, bf16)
  QT,KT [d, t] per head, 2 heads stacked per 128 partitions
  V_aug [t, 65] per head (col 64 = ones -> PV matmul emits softmax denom)
  S^T  [j, i] tiles from lhsT=KT, rhs=QT (K=64 contraction); the head pair's
       two S tiles land in one [128, 2, 512] PSUM tile (2 banks) so a single
       Exp activation serves both heads; the two matmuls are row-tiled
       (rows 0-63 / 64-127) and run concurrently on the PE.
  P = exp(S^T/8) (ScalarE, PSUM->SBUF bf16); causal diagonal tiles masked by
       a precomputed 0/1 multiply (VectorE); off-diagonal j>i tiles skipped.
  O_aug^T [65, i] accumulated over j chunks per head; row 64 = denominator.
  AT = O^T * (1/denom) broadcast -> proj lhsT; partial = A @ w_proj_slice.

Optimizations vs the 352us baseline (now ~287us):
  - diagonal narrowing: for diagonal j-tile r (= jt - 4*ci) only the
    i-range [128*r, 512) survives the causal mask; S matmul, Exp, mask
    mul and PV all restrict to that window.
  - consolidated priority-ordered DMA (each dma_start costs ~650ns of
    serial sync-engine time) with subtile deps for pipelined consumption.
  - QKV/proj production decomposed into 8-matmul units: all 16 V tiles
    up front (the PE workload while input DMA streams), Q/K chunk ci+1
    and proj chunk ci-1 emitted at the (ci, pair) boundaries; filler
    units draw PSUM from the o0/o1 tags so the "s" tag stays dedicated
    to the S->Exp double buffer.
  - warm-up matmuls on a scratch tile at t=0 fill the first-DMA wait and
    hold the PE HAM clock gate at full rate for the real work.
  - drain phase: proj 10/11 (ci2-dependent) bridge the last pair's
    normalization; tail proj copies split across VectorE/ScalarE with
    per-half DMA.
"""

import numpy as np
import ml_dtypes

B, T, C, H, D = 4, 2048, 1024, 16, 64
HL = H // 2          # heads per core
CL = HL * D          # local channels (512)
NPAIR = HL // 2      # head pairs per core (4)
CCH = C // 128       # contraction chunks for qkv (8)
PCH = CL // 128      # contraction chunks for proj (4)
TT = T // 128        # t tiles (16)
NI = T // 512        # i chunks (4)
N_CORES = 8
BF16 = ml_dtypes.bfloat16

_compiled = None


def _build(nc):
    import concourse.tile as tile
    from concourse import mybir

    bf = mybir.dt.bfloat16
    f32 = mybir.dt.float32
    Exp = mybir.ActivationFunctionType.Exp

    xT = nc.dram_tensor("xT", [C, T], bf, kind="ExternalInput").ap()
    wq = nc.dram_tensor("wq", [C, CL], bf, kind="ExternalInput").ap()
    wk = nc.dram_tensor("wk", [C, CL], bf, kind="ExternalInput").ap()
    wv = nc.dram_tensor("wv", [C, CL], bf, kind="ExternalInput").ap()
    bq = nc.dram_tensor("bq", [128, NPAIR], f32, kind="ExternalInput").ap()
    bk = nc.dram_tensor("bk", [128, NPAIR], f32, kind="ExternalInput").ap()
    bv = nc.dram_tensor("bv", [128, CL], f32, kind="ExternalInput").ap()
    wp = nc.dram_tensor("wp", [CL, C], bf, kind="ExternalInput").ap()
    out = nc.dram_tensor("out", [T, C], bf, kind="ExternalOutput").ap()

    xT_r = xT.rearrange("(cc p) t -> p cc t", p=128)
    wq_r = wq.rearrange("(cc p) m -> p cc m", p=128)
    wk_r = wk.rearrange("(cc p) m -> p cc m", p=128)
    wv_r = wv.rearrange("(cc p) m -> p cc m", p=128)
    wp_r = wp.rearrange("(cc p) n -> p cc n", p=128)

    with tile.TileContext(nc) as tc:
        import contextlib

        with contextlib.ExitStack() as ctx:
            persist = ctx.enter_context(tc.tile_pool(name="persist", bufs=1))
            # PSUM budget (8 banks): "s" [128,2,512] f32 = 2 banks x 2 bufs;
            # o0/o1 [65,512] f32 = 1 bank x 2 bufs each (the two bufs hold
            # the two in-flight head-pairs' accumulators). Filler units draw
            # 1-bank tiles from the o0/o1 tags so "s" stays dedicated to the
            # S->Exp pipeline.
            ps_pool = ctx.enter_context(tc.tile_pool(name="ps_pool", bufs=2, space="PSUM"))
            o_ps = ctx.enter_context(tc.tile_pool(name="o_ps", bufs=2, space="PSUM"))
            p_pool = ctx.enter_context(tc.tile_pool(name="p_pool", bufs=6))
            r_pool = ctx.enter_context(tc.tile_pool(name="r_pool", bufs=4))
            st_pool = ctx.enter_context(tc.tile_pool(name="st_pool", bufs=3))

            # ---- persistent SBUF tensors ----
            xT_sb = persist.tile([128, CCH, T], bf)
            wq_sb = persist.tile([128, CCH, CL], bf)
            wk_sb = persist.tile([128, CCH, CL], bf)
            wv_sb = persist.tile([128, CCH, CL], bf)
            wp_sb = persist.tile([128, PCH, C], bf)
            bq_sb = persist.tile([128, NPAIR], f32)
            bk_sb = persist.tile([128, NPAIR], f32)
            bv_sb = persist.tile([128, CL], f32)
            QT_sb = persist.tile([128, NPAIR, T], bf)
            KT_sb = persist.tile([128, NPAIR, T], bf)
            V_sb = persist.tile([128, TT, HL, D + 1], bf)
            AT_sb = persist.tile([128, PCH, T], bf)
            m_sb = persist.tile([128, 4, 2, 512], bf)

            # ---- PE warm-up: matmuls on a scratch tile, no DMA deps. They
            # fill the otherwise-idle first ~10us (input DMA latency) and
            # keep the HAM activity window busy so the first real matmuls
            # run at 2.4 GHz instead of 1.2. ----
            warm_sb = persist.tile([128, 512], bf)
            nc.vector.memset(warm_sb[:], 0.0)
            for w in range(6):
                wps = o_ps.tile([128, 512], f32, tag=("o0" if w % 2 == 0 else "o1"), name=f"warm{w}")
                for k in range(4):
                    nc.tensor.matmul(
                        warm_ps_out := wps[:],
                        lhsT=warm_sb[:, 0:128],
                        rhs=warm_sb[:],
                        start=(k == 0),
                        stop=(k == 3),
                    )

            # ---- DMA: few large transfers, priority order. The sync engine
            # issues each dma_start serially, so count matters; subtile deps
            # let consumers start as soon as their transfer lands. ----
            TH = T // 2
            nc.sync.dma_start(out=wv_sb[:, 0:2, :], in_=wv_r[:, 0:2, :])
            nc.sync.dma_start(out=xT_sb[:, 0:2, :TH], in_=xT_r[:, 0:2, :TH])
            nc.sync.dma_start(out=wv_sb[:, 2:8, :], in_=wv_r[:, 2:8, :])
            nc.sync.dma_start(out=xT_sb[:, 2:4, :TH], in_=xT_r[:, 2:4, :TH])
            nc.sync.dma_start(out=xT_sb[:, 4:8, :TH], in_=xT_r[:, 4:8, :TH])
            nc.sync.dma_start(out=bv_sb[:], in_=bv[:])
            nc.sync.dma_start(out=xT_sb[:, 0:4, TH:], in_=xT_r[:, 0:4, TH:])
            nc.sync.dma_start(out=xT_sb[:, 4:8, TH:], in_=xT_r[:, 4:8, TH:])
            nc.sync.dma_start(out=wq_sb[:, 0:4, :], in_=wq_r[:, 0:4, :])
            nc.sync.dma_start(out=wq_sb[:, 4:8, :], in_=wq_r[:, 4:8, :])
            nc.sync.dma_start(out=wk_sb[:, 0:4, :], in_=wk_r[:, 0:4, :])
            nc.sync.dma_start(out=wk_sb[:, 4:8, :], in_=wk_r[:, 4:8, :])
            nc.sync.dma_start(out=bq_sb[:], in_=bq[:])
            nc.sync.dma_start(out=bk_sb[:], in_=bk[:])
            nc.sync.dma_start(out=wp_sb[:, 0:2, :], in_=wp_r[:, 0:2, :])
            nc.sync.dma_start(out=wp_sb[:, 2:4, :], in_=wp_r[:, 2:4, :])

            # causal 0/1 masks, replicated for the pair dim:
            # m[r][jj, :, ii] = 1 if ii - jj >= 128*r else 0
            for r in range(4):
                nc.vector.memset(m_sb[:, r], 1.0)
                nc.gpsimd.affine_select(
                    out=m_sb[:, r],
                    in_=m_sb[:, r],
                    compare_op=mybir.AluOpType.is_ge,
                    fill=0.0,
                    base=-128 * r,
                    pattern=[[0, 2], [1, 512]],
                    channel_multiplier=-1,
                )
            # ones column of V_aug
            nc.vector.memset(V_sb[:, :, :, D], 1.0)

            # ---- filler units: 8-matmul groups, PSUM from o0/o1 tags ----
            def v_unit(tt, ftag):
                ps = o_ps.tile([128, 512], f32, tag=ftag, name=f"vps{tt}")
                for cc in range(CCH):
                    nc.tensor.matmul(
                        ps[:],
                        lhsT=xT_sb[:, cc, tt * 128 : (tt + 1) * 128],
                        rhs=wv_sb[:, cc, :],
                        start=(cc == 0),
                        stop=(cc == CCH - 1),
                    )
                nc.vector.tensor_add(
                    V_sb[:, tt, :, 0:D],
                    ps.rearrange("p (h d) -> p h d", h=HL),
                    bv_sb[:].rearrange("p (h d) -> p h d", h=HL),
                )

            def qk_unit(pair, which, tc_, ftag):
                w_sb, dst, b_sb = (
                    (wq_sb, QT_sb, bq_sb) if which == 0 else (wk_sb, KT_sb, bk_sb)
                )
                ps = o_ps.tile([128, 512], f32, tag=ftag, name=f"qkps{pair}_{which}_{tc_}")
                t0 = tc_ * 512
                for cc in range(CCH):
                    nc.tensor.matmul(
                        ps[:],
                        lhsT=w_sb[:, cc, pair * 128 : (pair + 1) * 128],
                        rhs=xT_sb[:, cc, t0 : t0 + 512],
                        start=(cc == 0),
                        stop=(cc == CCH - 1),
                    )
                nc.vector.tensor_scalar_add(
                    dst[:, pair, t0 : t0 + 512], ps[:], b_sb[:, pair : pair + 1]
                )

            def proj_unit(tt, tail=False):
                so = st_pool.tile([128, C], bf, tag="so", name=f"so{tt}")
                for nh in range(2):
                    ps = o_ps.tile(
                        [128, 512], f32, tag=("o0" if nh == 0 else "o1"), name=f"pps{tt}_{nh}"
                    )
                    for cc in range(PCH):
                        nc.tensor.matmul(
                            ps[:],
                            lhsT=AT_sb[:, cc, tt * 128 : (tt + 1) * 128],
                            rhs=wp_sb[:, cc, nh * 512 : (nh + 1) * 512],
                            start=(cc == 0),
                            stop=(cc == PCH - 1),
                        )
                    if tail and nh == 1:
                        # drain phase: VectorE is the bottleneck; use ScalarE
                        nc.scalar.activation(
                            so[:, nh * 512 : (nh + 1) * 512], ps[:],
                            mybir.ActivationFunctionType.Copy,
                        )
                    else:
                        nc.vector.tensor_copy(so[:, nh * 512 : (nh + 1) * 512], ps[:])
                    if tail:
                        nc.sync.dma_start(
                            out=out[tt * 128 : (tt + 1) * 128, nh * 512 : (nh + 1) * 512],
                            in_=so[:, nh * 512 : (nh + 1) * 512],
                        )
                if not tail:
                    nc.sync.dma_start(out=out[tt * 128 : (tt + 1) * 128, :], in_=so[:])

            # ---- pre-attention fillers: all 16 V tiles (the V matmuls are
            # the PE's workload while the input DMA streams in) and pair 0's
            # Q0/K0 ----
            for tt in range(16):
                v_unit(tt, "o0" if tt % 2 == 0 else "o1")
            qk_unit(0, 0, 0, "o0")
            qk_unit(0, 1, 0, "o1")

            # ---- attention, i-chunk outer; each pair's normalization and
            # filler units are deferred until the next pair's S->Exp pipeline
            # has restarted (after its jt=1 group) ----
            for ci in range(NI):
                njt = 4 * (ci + 1)
                for pair in range(NPAIR):
                    if ci == 0 and pair < NPAIR - 1:
                        qk_unit(pair + 1, 0, 0, "o0")
                        qk_unit(pair + 1, 1, 0, "o1")
                    o0 = o_ps.tile([D + 1, 512], f32, tag="o0", name=f"o0_{ci}_{pair}")
                    o1 = o_ps.tile([D + 1, 512], f32, tag="o1", name=f"o1_{ci}_{pair}")
                    for jt in range(njt):
                        r = jt - 4 * ci
                        i0 = 128 * r if r >= 0 else 0
                        st = ps_pool.tile([128, 2, 512], f32, tag="s", name=f"st{ci}_{pair}_{jt}")
                        for s in range(2):
                            nc.tensor.matmul(
                                st[:, s, i0:],
                                lhsT=KT_sb[
                                    64 * s : 64 * (s + 1),
                                    pair,
                                    jt * 128 : (jt + 1) * 128,
                                ],
                                rhs=QT_sb[
                                    64 * s : 64 * (s + 1),
                                    pair,
                                    ci * 512 + i0 : (ci + 1) * 512,
                                ],
                                start=True,
                                stop=True,
                            )
                        pt = p_pool.tile([128, 2, 512], bf, tag="p", name=f"pt{ci}_{pair}_{jt}")
                        nc.scalar.activation(pt[:, :, i0:], st[:, :, i0:], Exp, scale=0.125)
                        if r >= 0:
                            nc.vector.tensor_mul(
                                pt[:, :, i0:], pt[:, :, i0:], m_sb[:, r, :, i0:]
                            )
                        for s, ot in enumerate((o0, o1)):
                            nc.tensor.matmul(
                                ot[:, i0:],
                                lhsT=V_sb[:, jt, 2 * pair + s, :],
                                rhs=pt[:, s, i0:],
                                start=(jt == 0),
                                stop=(jt == njt - 1),
                            )
                    def boundary(ci=ci, pair=pair, o0=o0, o1=o1):
                        for s, ot in enumerate((o0, o1)):
                            rc = r_pool.tile([1, 512], f32, tag="rc", name=f"rc{ci}_{pair}_{s}")
                            dn = r_pool.tile([1, 512], f32, tag="dn", name=f"dn{ci}_{pair}_{s}")
                            if ci == NI - 1 and pair >= 2:
                                # drain phase: VectorE is the serial bottleneck
                                nc.scalar.activation(
                                    dn[:], ot[D : D + 1, :],
                                    mybir.ActivationFunctionType.Copy,
                                )
                            else:
                                nc.vector.tensor_copy(dn[:], ot[D : D + 1, :])
                            nc.vector.reciprocal_approx_fast(rc[:], dn[:])
                            rb = r_pool.tile([128, 512], f32, tag="rb", name=f"rb{ci}_{pair}_{s}")
                            nc.gpsimd.partition_broadcast(rb[:], rc[:])
                            nc.vector.tensor_mul(
                                AT_sb[
                                    64 * s : 64 * (s + 1),
                                    pair,
                                    ci * 512 : (ci + 1) * 512,
                                ],
                                ot[0:D, :],
                                rb[64 * s : 64 * (s + 1), :],
                            )
                        # just-in-time fillers for upcoming chunks / proj
                        if ci < NI - 1:
                            qk_unit(pair, 0, ci + 1, "o0")
                            qk_unit(pair, 1, ci + 1, "o1")
                        if ci >= 1 and not (ci == NI - 1 and pair >= 2):
                            proj_unit(4 * (ci - 1) + pair)
                    boundary()
            # proj 10/11 depend only on i-chunk 2: they overlap the last
            # pair's normalization chain and keep the PE (HAM) warm
            proj_unit(10)
            proj_unit(11)
            for tt in range(12, 16):
                proj_unit(tt, tail=True)
    return nc


def _get_compiled():
    global _compiled
    if _compiled is None:
        from concourse import bacc

        nc = bacc.Bacc(
            "TRN2", target_bir_lowering=False, debug=False, num_devices=N_CORES
        )
        _build(nc)
        nc.compile()
        _compiled = nc
    return _compiled


def _shard_inputs(x, w_qkv, b_qkv, w_proj):
    """Build the 8 per-core input dicts (host-side transpose/slice/cast)."""
    in_maps = []
    wq_f, wk_f, wv_f = w_qkv[:, :C], w_qkv[:, C : 2 * C], w_qkv[:, 2 * C :]
    for c in range(N_CORES):
        b, g = c // 2, c % 2
        sl = slice(g * CL, (g + 1) * CL)
        bqs = np.ascontiguousarray(b_qkv[0 * C :][sl].reshape(NPAIR, 128).T)
        bks = np.ascontiguousarray(b_qkv[1 * C :][sl].reshape(NPAIR, 128).T)
        bvs = np.ascontiguousarray(
            np.broadcast_to(b_qkv[2 * C :][sl][None, :], (128, CL))
        )
        in_maps.append(
            {
                "xT": np.ascontiguousarray(x[b].T).astype(BF16),
                "wq": np.ascontiguousarray(wq_f[:, sl]).astype(BF16),
                "wk": np.ascontiguousarray(wk_f[:, sl]).astype(BF16),
                "wv": np.ascontiguousarray(wv_f[:, sl]).astype(BF16),
                "bq": bqs.astype(np.float32),
                "bk": bks.astype(np.float32),
                "bv": bvs.astype(np.float32),
                "wp": np.ascontiguousarray(w_proj[sl, :]).astype(BF16),
            }
        )
    return in_maps


def kernel(x, w_qkv, b_qkv, w_proj, b_proj, _trace=False, _tmpdir=None):
    from concourse.bass_utils import run_bass_kernel_spmd

    x = np.asarray(x, dtype=np.float32)
    w_qkv = np.asarray(w_qkv, dtype=np.float32)
    b_qkv = np.asarray(b_qkv, dtype=np.float32)
    w_proj = np.asarray(w_proj, dtype=np.float32)
    b_proj = np.asarray(b_proj, dtype=np.float32)

    nc = _get_compiled()
    in_maps = _shard_inputs(x, w_qkv, b_qkv, w_proj)
    res = run_bass_kernel_spmd(
        nc,
        in_maps,
        core_ids=list(range(N_CORES)),
        trace=_trace,
        tmpdir=_tmpdir,
    )
    out = np.empty((B, T, C), dtype=np.float32)
    for b in range(B):
        out[b] = (
            res.results[2 * b]["out"].astype(np.float32)
            + res.results[2 * b + 1]["out"].astype(np.float32)
            + b_proj
        )
    kernel._last_result = res
    return out


# revision 12
# speedup vs baseline: 1.0035x; 1.0035x over previous
"""Causal self-attention on 8 Trainium2 NeuronCores.

Problem (hardcoded): B=4, T=2048, C=1024, H=16, D=64.
  qkv = x @ w_qkv + b_qkv ; per-head causal softmax attention ; out = attn @ w_proj + b_proj

Sharding (per hint): tensor-parallel over heads x data-parallel over batch.
  core c -> batch b = c // 2, head group g = c % 2 (heads g*8 .. g*8+7).
Each core computes QKV for its 8 heads, causal attention, and a partial
projection (its 512 input channels of w_proj). Host sums the two partials per
batch and adds b_proj.

On-core layout ("transposed" attention so softmax reduction lands on the
matmul contraction axis):
  xT   [C, T]  (host pre-transposed, bf16)
  QT,KT [d, t] per head, 2 heads stacked per 128 partitions
  V_aug [t, 65] per head (col 64 = ones -> PV matmul emits softmax denom)
  S^T  [j, i] tiles from lhsT=KT, rhs=QT (K=64 contraction); the head pair's
       two S tiles land in one [128, 2, 512] PSUM tile (2 banks) so a single
       Exp activation serves both heads; the two matmuls are row-tiled
       (rows 0-63 / 64-127) and run concurrently on the PE.
  P = exp(S^T/8) (ScalarE, PSUM->SBUF bf16); causal diagonal tiles masked by
       a precomputed 0/1 multiply (VectorE); off-diagonal j>i tiles skipped.
  O_aug^T [65, i] accumulated over j chunks per head; row 64 = denominator.
  AT = O^T * (1/denom) broadcast -> proj lhsT; partial = A @ w_proj_slice.

Optimizations vs the 352us baseline (now ~287us):
  - diagonal narrowing: for diagonal j-tile r (= jt - 4*ci) only the
    i-range [128*r, 512) survives the causal mask; S matmul, Exp, mask
    mul and PV all restrict to that window.
  - consolidated priority-ordered DMA (each dma_start costs ~650ns of
    serial sync-engine time) with subtile deps for pipelined consumption.
  - QKV/proj production decomposed into 8-matmul units: all 16 V tiles
    up front (the PE workload while input DMA streams), Q/K chunk ci+1
    and proj chunk ci-1 emitted at the (ci, pair) boundaries; filler
    units draw PSUM from the o0/o1 tags so the "s" tag stays dedicated
    to the S->Exp double buffer.
  - drain phase: proj 10/11 (ci2-dependent) bridge the last pair's
    normalization; tail proj copies split across VectorE/ScalarE with
    per-half DMA.
"""

import numpy as np
import ml_dtypes

B, T, C, H, D = 4, 2048, 1024, 16, 64
HL = H // 2          # heads per core
CL = HL * D          # local channels (512)
NPAIR = HL // 2      # head pairs per core (4)
CCH = C // 128       # contraction chunks for qkv (8)
PCH = CL // 128      # contraction chunks for proj (4)
TT = T // 128        # t tiles (16)
NI = T // 512        # i chunks (4)
N_CORES = 8
BF16 = ml_dtypes.bfloat16

_compiled = None


def _build(nc):
    import concourse.tile as tile
    from concourse import mybir

    bf = mybir.dt.bfloat16
    f32 = mybir.dt.float32
    Exp = mybir.ActivationFunctionType.Exp

    xT = nc.dram_tensor("xT", [C, T], bf, kind="ExternalInput").ap()
    wq = nc.dram_tensor("wq", [C, CL], bf, kind="ExternalInput").ap()
    wk = nc.dram_tensor("wk", [C, CL], bf, kind="ExternalInput").ap()
    wv = nc.dram_tensor("wv", [C, CL], bf, kind="ExternalInput").ap()
    bq = nc.dram_tensor("bq", [128, NPAIR], f32, kind="ExternalInput").ap()
    bk = nc.dram_tensor("bk", [128, NPAIR], f32, kind="ExternalInput").ap()
    bv = nc.dram_tensor("bv", [128, CL], f32, kind="ExternalInput").ap()
    wp = nc.dram_tensor("wp", [CL, C], bf, kind="ExternalInput").ap()
    out = nc.dram_tensor("out", [T, C], bf, kind="ExternalOutput").ap()

    xT_r = xT.rearrange("(cc p) t -> p cc t", p=128)
    wq_r = wq.rearrange("(cc p) m -> p cc m", p=128)
    wk_r = wk.rearrange("(cc p) m -> p cc m", p=128)
    wv_r = wv.rearrange("(cc p) m -> p cc m", p=128)
    wp_r = wp.rearrange("(cc p) n -> p cc n", p=128)

    with tile.TileContext(nc) as tc:
        import contextlib

        with contextlib.ExitStack() as ctx:
            persist = ctx.enter_context(tc.tile_pool(name="persist", bufs=1))
            # PSUM budget (8 banks): "s" [128,2,512] f32 = 2 banks x 2 bufs;
            # o0/o1 [65,512] f32 = 1 bank x 2 bufs each (the two bufs hold
            # the two in-flight head-pairs' accumulators). Filler units draw
            # 1-bank tiles from the o0/o1 tags so "s" stays dedicated to the
            # S->Exp pipeline.
            ps_pool = ctx.enter_context(tc.tile_pool(name="ps_pool", bufs=2, space="PSUM"))
            o_ps = ctx.enter_context(tc.tile_pool(name="o_ps", bufs=2, space="PSUM"))
            p_pool = ctx.enter_context(tc.tile_pool(name="p_pool", bufs=6))
            r_pool = ctx.enter_context(tc.tile_pool(name="r_pool", bufs=4))
            st_pool = ctx.enter_context(tc.tile_pool(name="st_pool", bufs=3))

            # ---- persistent SBUF tensors ----
            xT_sb = persist.tile([128, CCH, T], bf)
            wq_sb = persist.tile([128, CCH, CL], bf)
            wk_sb = persist.tile([128, CCH, CL], bf)
            wv_sb = persist.tile([128, CCH, CL], bf)
            wp_sb = persist.tile([128, PCH, C], bf)
            bq_sb = persist.tile([128, NPAIR], f32)
            bk_sb = persist.tile([128, NPAIR], f32)
            bv_sb = persist.tile([128, CL], f32)
            QT_sb = persist.tile([128, NPAIR, T], bf)
            KT_sb = persist.tile([128, NPAIR, T], bf)
            V_sb = persist.tile([128, TT, HL, D + 1], bf)
            AT_sb = persist.tile([128, PCH, T], bf)
            m_sb = persist.tile([128, 4, 2, 512], bf)

            # ---- DMA: few large transfers, priority order. The sync engine
            # issues each dma_start serially, so count matters; subtile deps
            # let consumers start as soon as their transfer lands. ----
            TH = T // 2
            nc.sync.dma_start(out=wv_sb[:, 0:2, :], in_=wv_r[:, 0:2, :])
            nc.sync.dma_start(out=xT_sb[:, 0:2, :TH], in_=xT_r[:, 0:2, :TH])
            nc.sync.dma_start(out=wv_sb[:, 2:8, :], in_=wv_r[:, 2:8, :])
            nc.sync.dma_start(out=xT_sb[:, 2:4, :TH], in_=xT_r[:, 2:4, :TH])
            nc.sync.dma_start(out=xT_sb[:, 4:8, :TH], in_=xT_r[:, 4:8, :TH])
            nc.sync.dma_start(out=bv_sb[:], in_=bv[:])
            nc.sync.dma_start(out=xT_sb[:, 0:4, TH:], in_=xT_r[:, 0:4, TH:])
            nc.sync.dma_start(out=xT_sb[:, 4:8, TH:], in_=xT_r[:, 4:8, TH:])
            nc.sync.dma_start(out=wq_sb[:, 0:4, :], in_=wq_r[:, 0:4, :])
            nc.sync.dma_start(out=wq_sb[:, 4:8, :], in_=wq_r[:, 4:8, :])
            nc.sync.dma_start(out=wk_sb[:, 0:4, :], in_=wk_r[:, 0:4, :])
            nc.sync.dma_start(out=wk_sb[:, 4:8, :], in_=wk_r[:, 4:8, :])
            nc.sync.dma_start(out=bq_sb[:], in_=bq[:])
            nc.sync.dma_start(out=bk_sb[:], in_=bk[:])
            nc.sync.dma_start(out=wp_sb[:, 0:2, :], in_=wp_r[:, 0:2, :])
            nc.sync.dma_start(out=wp_sb[:, 2:4, :], in_=wp_r[:, 2:4, :])

            # causal 0/1 masks, replicated for the pair dim:
            # m[r][jj, :, ii] = 1 if ii - jj >= 128*r else 0
            for r in range(4):
                nc.vector.memset(m_sb[:, r], 1.0)
                nc.gpsimd.affine_select(
                    out=m_sb[:, r],
                    in_=m_sb[:, r],
                    compare_op=mybir.AluOpType.is_ge,
                    fill=0.0,
                    base=-128 * r,
                    pattern=[[0, 2], [1, 512]],
                    channel_multiplier=-1,
                )
            # ones column of V_aug
            nc.vector.memset(V_sb[:, :, :, D], 1.0)

            # ---- filler units: 8-matmul groups, PSUM from o0/o1 tags ----
            def v_unit(tt, ftag):
                ps = o_ps.tile([128, 512], f32, tag=ftag, name=f"vps{tt}")
                for cc in range(CCH):
                    nc.tensor.matmul(
                        ps[:],
                        lhsT=xT_sb[:, cc, tt * 128 : (tt + 1) * 128],
                        rhs=wv_sb[:, cc, :],
                        start=(cc == 0),
                        stop=(cc == CCH - 1),
                    )
                nc.vector.tensor_add(
                    V_sb[:, tt, :, 0:D],
                    ps.rearrange("p (h d) -> p h d", h=HL),
                    bv_sb[:].rearrange("p (h d) -> p h d", h=HL),
                )

            def qk_unit(pair, which, tc_, ftag):
                w_sb, dst, b_sb = (
                    (wq_sb, QT_sb, bq_sb) if which == 0 else (wk_sb, KT_sb, bk_sb)
                )
                ps = o_ps.tile([128, 512], f32, tag=ftag, name=f"qkps{pair}_{which}_{tc_}")
                t0 = tc_ * 512
                for cc in range(CCH):
                    nc.tensor.matmul(
                        ps[:],
                        lhsT=w_sb[:, cc, pair * 128 : (pair + 1) * 128],
                        rhs=xT_sb[:, cc, t0 : t0 + 512],
                        start=(cc == 0),
                        stop=(cc == CCH - 1),
                    )
                nc.vector.tensor_scalar_add(
                    dst[:, pair, t0 : t0 + 512], ps[:], b_sb[:, pair : pair + 1]
                )

            def proj_unit(tt, tail=False):
                so = st_pool.tile([128, C], bf, tag="so", name=f"so{tt}")
                for nh in range(2):
                    ps = o_ps.tile(
                        [128, 512], f32, tag=("o0" if nh == 0 else "o1"), name=f"pps{tt}_{nh}"
                    )
                    for cc in range(PCH):
                        nc.tensor.matmul(
                            ps[:],
                            lhsT=AT_sb[:, cc, tt * 128 : (tt + 1) * 128],
                            rhs=wp_sb[:, cc, nh * 512 : (nh + 1) * 512],
                            start=(cc == 0),
                            stop=(cc == PCH - 1),
                        )
                    if tail and nh == 1:
                        # drain phase: VectorE is the bottleneck; use ScalarE
                        nc.scalar.activation(
                            so[:, nh * 512 : (nh + 1) * 512], ps[:],
                            mybir.ActivationFunctionType.Copy,
                        )
                    else:
                        nc.vector.tensor_copy(so[:, nh * 512 : (nh + 1) * 512], ps[:])
                    if tail:
                        nc.sync.dma_start(
                            out=out[tt * 128 : (tt + 1) * 128, nh * 512 : (nh + 1) * 512],
                            in_=so[:, nh * 512 : (nh + 1) * 512],
                        )
                if not tail:
                    nc.sync.dma_start(out=out[tt * 128 : (tt + 1) * 128, :], in_=so[:])

            # ---- pre-attention fillers: all 16 V tiles (the V matmuls are
            # the PE's workload while the input DMA streams in) and pair 0's
            # Q0/K0 ----
            for tt in range(16):
                v_unit(tt, "o0" if tt % 2 == 0 else "o1")
            qk_unit(0, 0, 0, "o0")
            qk_unit(0, 1, 0, "o1")

            # ---- attention, i-chunk outer; normalization and filler units
            # run at each (ci, pair) boundary ----
            for ci in range(NI):
                njt = 4 * (ci + 1)
                for pair in range(NPAIR):
                    if ci == 0 and pair < NPAIR - 1:
                        qk_unit(pair + 1, 0, 0, "o0")
                        qk_unit(pair + 1, 1, 0, "o1")
                    o0 = o_ps.tile([D + 1, 512], f32, tag="o0", name=f"o0_{ci}_{pair}")
                    o1 = o_ps.tile([D + 1, 512], f32, tag="o1", name=f"o1_{ci}_{pair}")
                    for jt in range(njt):
                        r = jt - 4 * ci
                        i0 = 128 * r if r >= 0 else 0
                        st = ps_pool.tile([128, 2, 512], f32, tag="s", name=f"st{ci}_{pair}_{jt}")
                        for s in range(2):
                            nc.tensor.matmul(
                                st[:, s, i0:],
                                lhsT=KT_sb[
                                    64 * s : 64 * (s + 1),
                                    pair,
                                    jt * 128 : (jt + 1) * 128,
                                ],
                                rhs=QT_sb[
                                    64 * s : 64 * (s + 1),
                                    pair,
                                    ci * 512 + i0 : (ci + 1) * 512,
                                ],
                                start=True,
                                stop=True,
                            )
                        pt = p_pool.tile([128, 2, 512], bf, tag="p", name=f"pt{ci}_{pair}_{jt}")
                        nc.scalar.activation(pt[:, :, i0:], st[:, :, i0:], Exp, scale=0.125)
                        if r >= 0:
                            nc.vector.tensor_mul(
                                pt[:, :, i0:], pt[:, :, i0:], m_sb[:, r, :, i0:]
                            )
                        for s, ot in enumerate((o0, o1)):
                            nc.tensor.matmul(
                                ot[:, i0:],
                                lhsT=V_sb[:, jt, 2 * pair + s, :],
                                rhs=pt[:, s, i0:],
                                start=(jt == 0),
                                stop=(jt == njt - 1),
                            )
                    def boundary(ci=ci, pair=pair, o0=o0, o1=o1):
                        for s, ot in enumerate((o0, o1)):
                            rc = r_pool.tile([1, 512], f32, tag="rc", name=f"rc{ci}_{pair}_{s}")
                            dn = r_pool.tile([1, 512], f32, tag="dn", name=f"dn{ci}_{pair}_{s}")
                            if ci == NI - 1 and pair >= 2:
                                # drain phase: VectorE is the serial bottleneck
                                nc.scalar.activation(
                                    dn[:], ot[D : D + 1, :],
                                    mybir.ActivationFunctionType.Copy,
                                )
                            else:
                                nc.vector.tensor_copy(dn[:], ot[D : D + 1, :])
                            nc.vector.reciprocal_approx_fast(rc[:], dn[:])
                            rb = r_pool.tile([128, 512], f32, tag="rb", name=f"rb{ci}_{pair}_{s}")
                            nc.gpsimd.partition_broadcast(rb[:], rc[:])
                            nc.vector.tensor_mul(
                                AT_sb[
                                    64 * s : 64 * (s + 1),
                                    pair,
                                    ci * 512 : (ci + 1) * 512,
                                ],
                                ot[0:D, :],
                                rb[64 * s : 64 * (s + 1), :],
                            )
                        # just-in-time fillers for upcoming chunks / proj
                        if ci < NI - 1:
                            qk_unit(pair, 0, ci + 1, "o0")
                            qk_unit(pair, 1, ci + 1, "o1")
                        if ci >= 1 and not (ci == NI - 1 and pair >= 2):
                            proj_unit(4 * (ci - 1) + pair)
                    boundary()
            # proj 10/11 depend only on i-chunk 2: they overlap the last
            # pair's normalization chain and keep the PE (HAM) warm
            proj_unit(10)
            proj_unit(11)
            for tt in range(12, 16):
                proj_unit(tt, tail=True)
    return nc


def _get_compiled():
    global _compiled
    if _compiled is None:
        from concourse import bacc

        nc = bacc.Bacc(
            "TRN2", target_bir_lowering=False, debug=False, num_devices=N_CORES
        )
        _build(nc)
        nc.compile()
        _compiled = nc
    return _compiled


def _shard_inputs(x, w_qkv, b_qkv, w_proj):
    """Build the 8 per-core input dicts (host-side transpose/slice/cast)."""
    in_maps = []
    wq_f, wk_f, wv_f = w_qkv[:, :C], w_qkv[:, C : 2 * C], w_qkv[:, 2 * C :]
    for c in range(N_CORES):
        b, g = c // 2, c % 2
        sl = slice(g * CL, (g + 1) * CL)
        bqs = np.ascontiguousarray(b_qkv[0 * C :][sl].reshape(NPAIR, 128).T)
        bks = np.ascontiguousarray(b_qkv[1 * C :][sl].reshape(NPAIR, 128).T)
        bvs = np.ascontiguousarray(
            np.broadcast_to(b_qkv[2 * C :][sl][None, :], (128, CL))
        )
        in_maps.append(
            {
                "xT": np.ascontiguousarray(x[b].T).astype(BF16),
                "wq": np.ascontiguousarray(wq_f[:, sl]).astype(BF16),
                "wk": np.ascontiguousarray(wk_f[:, sl]).astype(BF16),
                "wv": np.ascontiguousarray(wv_f[:, sl]).astype(BF16),
                "bq": bqs.astype(np.float32),
                "bk": bks.astype(np.float32),
                "bv": bvs.astype(np.float32),
                "wp": np.ascontiguousarray(w_proj[sl, :]).astype(BF16),
            }
        )
    return in_maps


def kernel(x, w_qkv, b_qkv, w_proj, b_proj, _trace=False, _tmpdir=None):
    from concourse.bass_utils import run_bass_kernel_spmd

    x = np.asarray(x, dtype=np.float32)
    w_qkv = np.asarray(w_qkv, dtype=np.float32)
    b_qkv = np.asarray(b_qkv, dtype=np.float32)
    w_proj = np.asarray(w_proj, dtype=np.float32)
    b_proj = np.asarray(b_proj, dtype=np.float32)

    nc = _get_compiled()
    in_maps = _shard_inputs(x, w_qkv, b_qkv, w_proj)
    res = run_bass_kernel_spmd(
        nc,
        in_maps,
        core_ids=list(range(N_CORES)),
        trace=_trace,
        tmpdir=_tmpdir,
    )
    out = np.empty((B, T, C), dtype=np.float32)
    for b in range(B):
        out[b] = (
            res.results[2 * b]["out"].astype(np.float32)
            + res.results[2 * b + 1]["out"].astype(np.float32)
            + b_proj
        )
    kernel._last_result = res
    return out


# revision 13
# speedup vs baseline: 1.0058x; 1.0023x over previous
"""Causal self-attention on 8 Trainium2 NeuronCores.

Problem (hardcoded): B=4, T=2048, C=1024, H=16, D=64.
  qkv = x @ w_qkv + b_qkv ; per-head causal softmax attention ; out = attn @ w_proj + b_proj

Sharding (per hint): tensor-parallel over heads x data-parallel over batch.
  core c -> batch b = c // 2, head group g = c % 2 (heads g*8 .. g*8+7).
Each core computes QKV for its 8 heads, causal attention, and a partial
projection (its 512 input channels of w_proj). Host sums the two partials per
batch and adds b_proj.

On-core layout ("transposed" attention so softmax reduction lands on the
matmul contraction axis):
  xT   [C, T]  (host pre-transposed, bf16)
  QT,KT [d, t] per head, 2 heads stacked per 128 partitions
  V_aug [t, 65] per head (col 64 = ones -> PV matmul emits softmax denom)
  S^T  [j, i] tiles from lhsT=KT, rhs=QT (K=64 contraction); the head pair's
       two S tiles land in one [128, 2, 512] PSUM tile (2 banks) so a single
       Exp activation serves both heads; the two matmuls are row-tiled
       (rows 0-63 / 64-127) and run concurrently on the PE.
  P = exp(S^T/8) (ScalarE, PSUM->SBUF bf16); causal diagonal tiles masked by
       a precomputed 0/1 multiply (VectorE); off-diagonal j>i tiles skipped.
  O_aug^T [65, i] accumulated over j chunks per head; row 64 = denominator.
  AT = O^T * (1/denom) broadcast -> proj lhsT; partial = A @ w_proj_slice.

Optimizations vs the 352us baseline (now ~287us):
  - diagonal narrowing: for diagonal j-tile r (= jt - 4*ci) only the
    i-range [128*r, 512) survives the causal mask; S matmul, Exp, mask
    mul and PV all restrict to that window.
  - consolidated priority-ordered DMA (each dma_start costs ~650ns of
    serial sync-engine time) with subtile deps for pipelined consumption.
  - QKV/proj production decomposed into 8-matmul units: all 16 V tiles
    up front (the PE workload while input DMA streams), Q/K chunk ci+1
    and proj chunk ci-1 emitted at the (ci, pair) boundaries; filler
    units draw PSUM from the o0/o1 tags so the "s" tag stays dedicated
    to the S->Exp double buffer.
  - drain phase: proj 10/11 (ci2-dependent) bridge the last pair's
    normalization; tail proj copies split across VectorE/ScalarE with
    per-half DMA.
"""

import numpy as np
import ml_dtypes

B, T, C, H, D = 4, 2048, 1024, 16, 64
HL = H // 2          # heads per core
CL = HL * D          # local channels (512)
NPAIR = HL // 2      # head pairs per core (4)
CCH = C // 128       # contraction chunks for qkv (8)
PCH = CL // 128      # contraction chunks for proj (4)
TT = T // 128        # t tiles (16)
NI = T // 512        # i chunks (4)
N_CORES = 8
BF16 = ml_dtypes.bfloat16

_compiled = None


def _build(nc):
    import concourse.tile as tile
    from concourse import mybir

    bf = mybir.dt.bfloat16
    f32 = mybir.dt.float32
    Exp = mybir.ActivationFunctionType.Exp

    xT = nc.dram_tensor("xT", [C, T], bf, kind="ExternalInput").ap()
    wq = nc.dram_tensor("wq", [C, CL], bf, kind="ExternalInput").ap()
    wk = nc.dram_tensor("wk", [C, CL], bf, kind="ExternalInput").ap()
    wv = nc.dram_tensor("wv", [C, CL], bf, kind="ExternalInput").ap()
    bq = nc.dram_tensor("bq", [128, NPAIR], f32, kind="ExternalInput").ap()
    bk = nc.dram_tensor("bk", [128, NPAIR], f32, kind="ExternalInput").ap()
    bv = nc.dram_tensor("bv", [128, CL], f32, kind="ExternalInput").ap()
    wp = nc.dram_tensor("wp", [CL, C], bf, kind="ExternalInput").ap()
    out = nc.dram_tensor("out", [T, C], bf, kind="ExternalOutput").ap()

    xT_r = xT.rearrange("(cc p) t -> p cc t", p=128)
    wq_r = wq.rearrange("(cc p) m -> p cc m", p=128)
    wk_r = wk.rearrange("(cc p) m -> p cc m", p=128)
    wv_r = wv.rearrange("(cc p) m -> p cc m", p=128)
    wp_r = wp.rearrange("(cc p) n -> p cc n", p=128)

    with tile.TileContext(nc) as tc:
        import contextlib

        with contextlib.ExitStack() as ctx:
            persist = ctx.enter_context(tc.tile_pool(name="persist", bufs=1))
            # PSUM budget (8 banks): "s" [128,2,512] f32 = 2 banks x 2 bufs;
            # o0/o1 [65,512] f32 = 1 bank x 2 bufs each (the two bufs hold
            # the two in-flight head-pairs' accumulators). Filler units draw
            # 1-bank tiles from the o0/o1 tags so "s" stays dedicated to the
            # S->Exp pipeline.
            ps_pool = ctx.enter_context(tc.tile_pool(name="ps_pool", bufs=2, space="PSUM"))
            o_ps = ctx.enter_context(tc.tile_pool(name="o_ps", bufs=2, space="PSUM"))
            p_pool = ctx.enter_context(tc.tile_pool(name="p_pool", bufs=6))
            r_pool = ctx.enter_context(tc.tile_pool(name="r_pool", bufs=4))
            st_pool = ctx.enter_context(tc.tile_pool(name="st_pool", bufs=3))

            # ---- persistent SBUF tensors ----
            xT_sb = persist.tile([128, CCH, T], bf)
            wq_sb = persist.tile([128, CCH, CL], bf)
            wk_sb = persist.tile([128, CCH, CL], bf)
            wv_sb = persist.tile([128, CCH, CL], bf)
            wp_sb = persist.tile([128, PCH, C], bf)
            bq_sb = persist.tile([128, NPAIR], f32)
            bk_sb = persist.tile([128, NPAIR], f32)
            bv_sb = persist.tile([128, CL], f32)
            QT_sb = persist.tile([128, NPAIR, T], bf)
            KT_sb = persist.tile([128, NPAIR, T], bf)
            V_sb = persist.tile([128, TT, HL, D + 1], bf)
            AT_sb = persist.tile([128, PCH, T], bf)
            m_sb = persist.tile([128, 4, 2, 512], bf)

            # ---- DMA: few large transfers, priority order. The sync engine
            # issues each dma_start serially, so count matters; subtile deps
            # let consumers start as soon as their transfer lands. ----
            TH = T // 2
            nc.sync.dma_start(out=wv_sb[:, 0:2, :], in_=wv_r[:, 0:2, :])
            nc.sync.dma_start(out=xT_sb[:, 0:2, :TH], in_=xT_r[:, 0:2, :TH])
            nc.sync.dma_start(out=wv_sb[:, 2:8, :], in_=wv_r[:, 2:8, :])
            nc.sync.dma_start(out=xT_sb[:, 2:4, :TH], in_=xT_r[:, 2:4, :TH])
            nc.sync.dma_start(out=xT_sb[:, 4:8, :TH], in_=xT_r[:, 4:8, :TH])
            nc.sync.dma_start(out=bv_sb[:], in_=bv[:])
            nc.sync.dma_start(out=xT_sb[:, 0:4, TH:], in_=xT_r[:, 0:4, TH:])
            nc.sync.dma_start(out=xT_sb[:, 4:8, TH:], in_=xT_r[:, 4:8, TH:])
            nc.sync.dma_start(out=wq_sb[:, 0:4, :], in_=wq_r[:, 0:4, :])
            nc.sync.dma_start(out=wq_sb[:, 4:8, :], in_=wq_r[:, 4:8, :])
            nc.sync.dma_start(out=wk_sb[:, 0:4, :], in_=wk_r[:, 0:4, :])
            nc.sync.dma_start(out=wk_sb[:, 4:8, :], in_=wk_r[:, 4:8, :])
            nc.sync.dma_start(out=bq_sb[:], in_=bq[:])
            nc.sync.dma_start(out=bk_sb[:], in_=bk[:])
            nc.sync.dma_start(out=wp_sb[:, 0:2, :], in_=wp_r[:, 0:2, :])
            nc.sync.dma_start(out=wp_sb[:, 2:4, :], in_=wp_r[:, 2:4, :])

            # causal 0/1 masks, replicated for the pair dim:
            # m[r][jj, :, ii] = 1 if ii - jj >= 128*r else 0
            for r in range(4):
                nc.vector.memset(m_sb[:, r], 1.0)
                nc.gpsimd.affine_select(
                    out=m_sb[:, r],
                    in_=m_sb[:, r],
                    compare_op=mybir.AluOpType.is_ge,
                    fill=0.0,
                    base=-128 * r,
                    pattern=[[0, 2], [1, 512]],
                    channel_multiplier=-1,
                )
            # ones column of V_aug
            nc.vector.memset(V_sb[:, :, :, D], 1.0)

            # ---- filler units: 8-matmul groups, PSUM from o0/o1 tags ----
            def v_unit(tt, ftag):
                ps = o_ps.tile([128, 512], f32, tag=ftag, name=f"vps{tt}")
                for cc in range(CCH):
                    nc.tensor.matmul(
                        ps[:],
                        lhsT=xT_sb[:, cc, tt * 128 : (tt + 1) * 128],
                        rhs=wv_sb[:, cc, :],
                        start=(cc == 0),
                        stop=(cc == CCH - 1),
                    )
                nc.vector.tensor_add(
                    V_sb[:, tt, :, 0:D],
                    ps.rearrange("p (h d) -> p h d", h=HL),
                    bv_sb[:].rearrange("p (h d) -> p h d", h=HL),
                )

            def qk_unit(pair, which, tc_, ftag):
                w_sb, dst, b_sb = (
                    (wq_sb, QT_sb, bq_sb) if which == 0 else (wk_sb, KT_sb, bk_sb)
                )
                ps = o_ps.tile([128, 512], f32, tag=ftag, name=f"qkps{pair}_{which}_{tc_}")
                t0 = tc_ * 512
                for cc in range(CCH):
                    nc.tensor.matmul(
                        ps[:],
                        lhsT=w_sb[:, cc, pair * 128 : (pair + 1) * 128],
                        rhs=xT_sb[:, cc, t0 : t0 + 512],
                        start=(cc == 0),
                        stop=(cc == CCH - 1),
                    )
                nc.vector.tensor_scalar_add(
                    dst[:, pair, t0 : t0 + 512], ps[:], b_sb[:, pair : pair + 1]
                )

            def proj_unit(tt, tail=False):
                so = st_pool.tile([128, C], bf, tag="so", name=f"so{tt}")
                for nh in range(2):
                    ps = o_ps.tile(
                        [128, 512], f32, tag=("o0" if nh == 0 else "o1"), name=f"pps{tt}_{nh}"
                    )
                    for cc in range(PCH):
                        nc.tensor.matmul(
                            ps[:],
                            lhsT=AT_sb[:, cc, tt * 128 : (tt + 1) * 128],
                            rhs=wp_sb[:, cc, nh * 512 : (nh + 1) * 512],
                            start=(cc == 0),
                            stop=(cc == PCH - 1),
                        )
                    if tail and nh == 1:
                        # drain phase: VectorE is the bottleneck; use ScalarE
                        nc.scalar.activation(
                            so[:, nh * 512 : (nh + 1) * 512], ps[:],
                            mybir.ActivationFunctionType.Copy,
                        )
                    else:
                        nc.vector.tensor_copy(so[:, nh * 512 : (nh + 1) * 512], ps[:])
                    if tail:
                        nc.sync.dma_start(
                            out=out[tt * 128 : (tt + 1) * 128, nh * 512 : (nh + 1) * 512],
                            in_=so[:, nh * 512 : (nh + 1) * 512],
                        )
                if not tail:
                    nc.sync.dma_start(out=out[tt * 128 : (tt + 1) * 128, :], in_=so[:])

            # ---- pre-attention fillers: all 16 V tiles (the V matmuls are
            # the PE's workload while the input DMA streams in) and pair 0's
            # Q0/K0 ----
            for tt in range(16):
                v_unit(tt, "o0" if tt % 2 == 0 else "o1")
            qk_unit(0, 0, 0, "o0")
            qk_unit(0, 1, 0, "o1")

            # ---- attention, i-chunk outer; normalization and filler units
            # run at each (ci, pair) boundary ----
            seq = [(ci, pair) for ci in range(NI) for pair in range(NPAIR)]
            o_next = None
            for idx, (ci, pair) in enumerate(seq):
                njt = 4 * (ci + 1)
                if True:
                    if ci == 0 and pair < NPAIR - 1:
                        qk_unit(pair + 1, 0, 0, "o0")
                        qk_unit(pair + 1, 1, 0, "o1")
                    if o_next is None:
                        o0 = o_ps.tile([D + 1, 512], f32, tag="o0", name=f"o0_{ci}_{pair}")
                        o1 = o_ps.tile([D + 1, 512], f32, tag="o1", name=f"o1_{ci}_{pair}")
                    else:
                        o0, o1 = o_next
                    for jt in range(njt):
                        r = jt - 4 * ci
                        i0 = 128 * r if r >= 0 else 0
                        st = ps_pool.tile([128, 2, 512], f32, tag="s", name=f"st{ci}_{pair}_{jt}")
                        for s in range(2):
                            nc.tensor.matmul(
                                st[:, s, i0:],
                                lhsT=KT_sb[
                                    64 * s : 64 * (s + 1),
                                    pair,
                                    jt * 128 : (jt + 1) * 128,
                                ],
                                rhs=QT_sb[
                                    64 * s : 64 * (s + 1),
                                    pair,
                                    ci * 512 + i0 : (ci + 1) * 512,
                                ],
                                start=True,
                                stop=True,
                            )
                        pt = p_pool.tile([128, 2, 512], bf, tag="p", name=f"pt{ci}_{pair}_{jt}")
                        nc.scalar.activation(pt[:, :, i0:], st[:, :, i0:], Exp, scale=0.125)
                        if r >= 0:
                            nc.vector.tensor_mul(
                                pt[:, :, i0:], pt[:, :, i0:], m_sb[:, r, :, i0:]
                            )
                        for s, ot in enumerate((o0, o1)):
                            nc.tensor.matmul(
                                ot[:, i0:],
                                lhsT=V_sb[:, jt, 2 * pair + s, :],
                                rhs=pt[:, s, i0:],
                                start=(jt == 0),
                                stop=(jt == njt - 1),
                            )
                    def boundary(ci=ci, pair=pair, o0=o0, o1=o1):
                        for s, ot in enumerate((o0, o1)):
                            rc = r_pool.tile([1, 512], f32, tag="rc", name=f"rc{ci}_{pair}_{s}")
                            dn = r_pool.tile([1, 512], f32, tag="dn", name=f"dn{ci}_{pair}_{s}")
                            if ci == NI - 1 and pair >= 2:
                                # drain phase: VectorE is the serial bottleneck
                                nc.scalar.activation(
                                    dn[:], ot[D : D + 1, :],
                                    mybir.ActivationFunctionType.Copy,
                                )
                            else:
                                nc.vector.tensor_copy(dn[:], ot[D : D + 1, :])
                            nc.vector.reciprocal_approx_fast(rc[:], dn[:])
                            rb = r_pool.tile([128, 512], f32, tag="rb", name=f"rb{ci}_{pair}_{s}")
                            nc.gpsimd.partition_broadcast(rb[:], rc[:])
                            nc.vector.tensor_mul(
                                AT_sb[
                                    64 * s : 64 * (s + 1),
                                    pair,
                                    ci * 512 : (ci + 1) * 512,
                                ],
                                ot[0:D, :],
                                rb[64 * s : 64 * (s + 1), :],
                            )
                    boundary()
                    # hoist the next pair's o-psum allocations ahead of the
                    # boundary fillers: keeps the accumulator rotation
                    # distance at 2 so the next pair's PV does not wait on
                    # this pair's normalization
                    if idx + 1 < len(seq):
                        nci, npair = seq[idx + 1]
                        o_next = (
                            o_ps.tile([D + 1, 512], f32, tag="o0", name=f"o0_{nci}_{npair}"),
                            o_ps.tile([D + 1, 512], f32, tag="o1", name=f"o1_{nci}_{npair}"),
                        )
                    # just-in-time fillers for upcoming chunks / proj drain
                    if ci < NI - 1:
                        qk_unit(pair, 0, ci + 1, "o0")
                        qk_unit(pair, 1, ci + 1, "o1")
                    if ci >= 1 and not (ci == NI - 1 and pair >= 2):
                        proj_unit(4 * (ci - 1) + pair)
            # proj 10/11 depend only on i-chunk 2: they overlap the last
            # pair's normalization chain and keep the PE (HAM) warm
            proj_unit(10)
            proj_unit(11)
            for tt in range(12, 16):
                proj_unit(tt, tail=True)
    return nc


def _get_compiled():
    global _compiled
    if _compiled is None:
        from concourse import bacc

        nc = bacc.Bacc(
            "TRN2", target_bir_lowering=False, debug=False, num_devices=N_CORES
        )
        _build(nc)
        nc.compile()
        _compiled = nc
    return _compiled


def _shard_inputs(x, w_qkv, b_qkv, w_proj):
    """Build the 8 per-core input dicts (host-side transpose/slice/cast)."""
    in_maps = []
    wq_f, wk_f, wv_f = w_qkv[:, :C], w_qkv[:, C : 2 * C], w_qkv[:, 2 * C :]
    for c in range(N_CORES):
        b, g = c // 2, c % 2
        sl = slice(g * CL, (g + 1) * CL)
        bqs = np.ascontiguousarray(b_qkv[0 * C :][sl].reshape(NPAIR, 128).T)
        bks = np.ascontiguousarray(b_qkv[1 * C :][sl].reshape(NPAIR, 128).T)
        bvs = np.ascontiguousarray(
            np.broadcast_to(b_qkv[2 * C :][sl][None, :], (128, CL))
        )
        in_maps.append(
            {
                "xT": np.ascontiguousarray(x[b].T).astype(BF16),
                "wq": np.ascontiguousarray(wq_f[:, sl]).astype(BF16),
                "wk": np.ascontiguousarray(wk_f[:, sl]).astype(BF16),
                "wv": np.ascontiguousarray(wv_f[:, sl]).astype(BF16),
                "bq": bqs.astype(np.float32),
                "bk": bks.astype(np.float32),
                "bv": bvs.astype(np.float32),
                "wp": np.ascontiguousarray(w_proj[sl, :]).astype(BF16),
            }
        )
    return in_maps


def kernel(x, w_qkv, b_qkv, w_proj, b_proj, _trace=False, _tmpdir=None):
    from concourse.bass_utils import run_bass_kernel_spmd

    x = np.asarray(x, dtype=np.float32)
    w_qkv = np.asarray(w_qkv, dtype=np.float32)
    b_qkv = np.asarray(b_qkv, dtype=np.float32)
    w_proj = np.asarray(w_proj, dtype=np.float32)
    b_proj = np.asarray(b_proj, dtype=np.float32)

    nc = _get_compiled()
    in_maps = _shard_inputs(x, w_qkv, b_qkv, w_proj)
    res = run_bass_kernel_spmd(
        nc,
        in_maps,
        core_ids=list(range(N_CORES)),
        trace=_trace,
        tmpdir=_tmpdir,
    )
    out = np.empty((B, T, C), dtype=np.float32)
    for b in range(B):
        out[b] = (
            res.results[2 * b]["out"].astype(np.float32)
            + res.results[2 * b + 1]["out"].astype(np.float32)
            + b_proj
        )
    kernel._last_result = res
    return out


# revision 14
# speedup vs baseline: 1.0105x; 1.0046x over previous
"""Causal self-attention on 8 Trainium2 NeuronCores.

Problem (hardcoded): B=4, T=2048, C=1024, H=16, D=64.
  qkv = x @ w_qkv + b_qkv ; per-head causal softmax attention ; out = attn @ w_proj + b_proj

Sharding (per hint): tensor-parallel over heads x data-parallel over batch.
  core c -> batch b = c // 2, head group g = c % 2 (heads g*8 .. g*8+7).
Each core computes QKV for its 8 heads, causal attention, and a partial
projection (its 512 input channels of w_proj). Host sums the two partials per
batch and adds b_proj.

On-core layout ("transposed" attention so softmax reduction lands on the
matmul contraction axis):
  xT   [C, T]  (host pre-transposed, bf16)
  QT,KT [d, t] per head, 2 heads stacked per 128 partitions
  V_aug [t, 65] per head (col 64 = ones -> PV matmul emits softmax denom)
  S^T  [j, i] tiles from lhsT=KT, rhs=QT (K=64 contraction); the head pair's
       two S tiles land in one [128, 2, 512] PSUM tile (2 banks) so a single
       Exp activation serves both heads; the two matmuls are row-tiled
       (rows 0-63 / 64-127) and run concurrently on the PE.
  P = exp(S^T/8) (ScalarE, PSUM->SBUF bf16); causal diagonal tiles masked by
       a precomputed 0/1 multiply (VectorE); off-diagonal j>i tiles skipped.
  O_aug^T [65, i] accumulated over j chunks per head; row 64 = denominator.
  AT = O^T * (1/denom) broadcast -> proj lhsT; partial = A @ w_proj_slice.

Optimizations vs the 352us baseline (now ~287us):
  - diagonal narrowing: for diagonal j-tile r (= jt - 4*ci) only the
    i-range [128*r, 512) survives the causal mask; S matmul, Exp, mask
    mul and PV all restrict to that window.
  - consolidated priority-ordered DMA (each dma_start costs ~650ns of
    serial sync-engine time) with subtile deps for pipelined consumption.
  - QKV/proj production decomposed into 8-matmul units: all 16 V tiles
    up front (the PE workload while input DMA streams), Q/K chunk ci+1
    and proj chunk ci-1 emitted at the (ci, pair) boundaries; filler
    units draw PSUM from the o0/o1 tags so the "s" tag stays dedicated
    to the S->Exp double buffer.
  - drain phase: proj 10/11 (ci2-dependent) bridge the last pair's
    normalization; tail proj copies split across VectorE/ScalarE with
    per-half DMA.
"""

import numpy as np
import ml_dtypes

B, T, C, H, D = 4, 2048, 1024, 16, 64
HL = H // 2          # heads per core
CL = HL * D          # local channels (512)
NPAIR = HL // 2      # head pairs per core (4)
CCH = C // 128       # contraction chunks for qkv (8)
PCH = CL // 128      # contraction chunks for proj (4)
TT = T // 128        # t tiles (16)
NI = T // 512        # i chunks (4)
N_CORES = 8
BF16 = ml_dtypes.bfloat16

_compiled = None


def _build(nc):
    import concourse.tile as tile
    from concourse import mybir

    bf = mybir.dt.bfloat16
    f32 = mybir.dt.float32
    Exp = mybir.ActivationFunctionType.Exp

    xT = nc.dram_tensor("xT", [C, T], bf, kind="ExternalInput").ap()
    wq = nc.dram_tensor("wq", [C, CL], bf, kind="ExternalInput").ap()
    wk = nc.dram_tensor("wk", [C, CL], bf, kind="ExternalInput").ap()
    wv = nc.dram_tensor("wv", [C, CL], bf, kind="ExternalInput").ap()
    bq = nc.dram_tensor("bq", [128, NPAIR], f32, kind="ExternalInput").ap()
    bk = nc.dram_tensor("bk", [128, NPAIR], f32, kind="ExternalInput").ap()
    bv = nc.dram_tensor("bv", [128, CL], f32, kind="ExternalInput").ap()
    wp = nc.dram_tensor("wp", [CL, C], bf, kind="ExternalInput").ap()
    out = nc.dram_tensor("out", [T, C], bf, kind="ExternalOutput").ap()

    xT_r = xT.rearrange("(cc p) t -> p cc t", p=128)
    wq_r = wq.rearrange("(cc p) m -> p cc m", p=128)
    wk_r = wk.rearrange("(cc p) m -> p cc m", p=128)
    wv_r = wv.rearrange("(cc p) m -> p cc m", p=128)
    wp_r = wp.rearrange("(cc p) n -> p cc n", p=128)

    with tile.TileContext(nc) as tc:
        import contextlib

        with contextlib.ExitStack() as ctx:
            persist = ctx.enter_context(tc.tile_pool(name="persist", bufs=1))
            # PSUM budget (8 banks): "s" [128,2,512] f32 = 2 banks x 2 bufs;
            # o0/o1 [65,512] f32 = 1 bank x 2 bufs each (the two bufs hold
            # the two in-flight head-pairs' accumulators). Filler units draw
            # 1-bank tiles from the o0/o1 tags so "s" stays dedicated to the
            # S->Exp pipeline.
            ps_pool = ctx.enter_context(tc.tile_pool(name="ps_pool", bufs=2, space="PSUM"))
            o_ps = ctx.enter_context(tc.tile_pool(name="o_ps", bufs=2, space="PSUM"))
            p_pool = ctx.enter_context(tc.tile_pool(name="p_pool", bufs=6))
            r_pool = ctx.enter_context(tc.tile_pool(name="r_pool", bufs=4))
            st_pool = ctx.enter_context(tc.tile_pool(name="st_pool", bufs=3))

            # ---- persistent SBUF tensors ----
            xT_sb = persist.tile([128, CCH, T], bf)
            wq_sb = persist.tile([128, CCH, CL], bf)
            wk_sb = persist.tile([128, CCH, CL], bf)
            wv_sb = persist.tile([128, CCH, CL], bf)
            wp_sb = persist.tile([128, PCH, C], bf)
            bq_sb = persist.tile([128, NPAIR], f32)
            bk_sb = persist.tile([128, NPAIR], f32)
            bv_sb = persist.tile([128, CL], f32)
            QT_sb = persist.tile([128, NPAIR, T], bf)
            KT_sb = persist.tile([128, NPAIR, T], bf)
            V_sb = persist.tile([128, TT, HL, D + 1], bf)
            AT_sb = persist.tile([128, PCH, T], bf)
            m_sb = persist.tile([128, 4, 2, 512], bf)

            # ---- DMA: few large transfers, priority order. The sync engine
            # issues each dma_start serially, so count matters; subtile deps
            # let consumers start as soon as their transfer lands. ----
            TH = T // 2
            nc.sync.dma_start(out=wv_sb[:, 0:2, :], in_=wv_r[:, 0:2, :])
            nc.sync.dma_start(out=wv_sb[:, 2:8, :], in_=wv_r[:, 2:8, :])
            nc.sync.dma_start(out=xT_sb[:, 0:2, :TH], in_=xT_r[:, 0:2, :TH])
            nc.sync.dma_start(out=xT_sb[:, 2:4, :TH], in_=xT_r[:, 2:4, :TH])
            nc.sync.dma_start(out=xT_sb[:, 4:8, :TH], in_=xT_r[:, 4:8, :TH])
            nc.sync.dma_start(out=bv_sb[:], in_=bv[:])
            nc.sync.dma_start(out=xT_sb[:, 0:4, TH:], in_=xT_r[:, 0:4, TH:])
            nc.sync.dma_start(out=xT_sb[:, 4:8, TH:], in_=xT_r[:, 4:8, TH:])
            nc.sync.dma_start(out=wq_sb[:, 0:4, :], in_=wq_r[:, 0:4, :])
            nc.sync.dma_start(out=wq_sb[:, 4:8, :], in_=wq_r[:, 4:8, :])
            nc.sync.dma_start(out=wk_sb[:, 0:4, :], in_=wk_r[:, 0:4, :])
            nc.sync.dma_start(out=wk_sb[:, 4:8, :], in_=wk_r[:, 4:8, :])
            nc.sync.dma_start(out=bq_sb[:], in_=bq[:])
            nc.sync.dma_start(out=bk_sb[:], in_=bk[:])
            nc.sync.dma_start(out=wp_sb[:, 0:2, :], in_=wp_r[:, 0:2, :])
            nc.sync.dma_start(out=wp_sb[:, 2:4, :], in_=wp_r[:, 2:4, :])

            # causal 0/1 masks, replicated for the pair dim:
            # m[r][jj, :, ii] = 1 if ii - jj >= 128*r else 0
            for r in range(4):
                nc.vector.memset(m_sb[:, r], 1.0)
                nc.gpsimd.affine_select(
                    out=m_sb[:, r],
                    in_=m_sb[:, r],
                    compare_op=mybir.AluOpType.is_ge,
                    fill=0.0,
                    base=-128 * r,
                    pattern=[[0, 2], [1, 512]],
                    channel_multiplier=-1,
                )
            # ones column of V_aug
            nc.vector.memset(V_sb[:, :, :, D], 1.0)

            # ---- filler units: 8-matmul groups, PSUM from o0/o1 tags ----
            def v_unit(tt, ftag):
                ps = o_ps.tile([128, 512], f32, tag=ftag, name=f"vps{tt}")
                for cc in range(CCH):
                    nc.tensor.matmul(
                        ps[:],
                        lhsT=xT_sb[:, cc, tt * 128 : (tt + 1) * 128],
                        rhs=wv_sb[:, cc, :],
                        start=(cc == 0),
                        stop=(cc == CCH - 1),
                    )
                nc.vector.tensor_add(
                    V_sb[:, tt, :, 0:D],
                    ps.rearrange("p (h d) -> p h d", h=HL),
                    bv_sb[:].rearrange("p (h d) -> p h d", h=HL),
                )

            def qk_unit(pair, which, tc_, ftag):
                w_sb, dst, b_sb = (
                    (wq_sb, QT_sb, bq_sb) if which == 0 else (wk_sb, KT_sb, bk_sb)
                )
                ps = o_ps.tile([128, 512], f32, tag=ftag, name=f"qkps{pair}_{which}_{tc_}")
                t0 = tc_ * 512
                for cc in range(CCH):
                    nc.tensor.matmul(
                        ps[:],
                        lhsT=w_sb[:, cc, pair * 128 : (pair + 1) * 128],
                        rhs=xT_sb[:, cc, t0 : t0 + 512],
                        start=(cc == 0),
                        stop=(cc == CCH - 1),
                    )
                nc.vector.tensor_scalar_add(
                    dst[:, pair, t0 : t0 + 512], ps[:], b_sb[:, pair : pair + 1]
                )

            def proj_unit(tt, tail=False):
                so = st_pool.tile([128, C], bf, tag="so", name=f"so{tt}")
                for nh in range(2):
                    ps = o_ps.tile(
                        [128, 512], f32, tag=("o0" if nh == 0 else "o1"), name=f"pps{tt}_{nh}"
                    )
                    for cc in range(PCH):
                        nc.tensor.matmul(
                            ps[:],
                            lhsT=AT_sb[:, cc, tt * 128 : (tt + 1) * 128],
                            rhs=wp_sb[:, cc, nh * 512 : (nh + 1) * 512],
                            start=(cc == 0),
                            stop=(cc == PCH - 1),
                        )
                    if tail and nh == 1:
                        # drain phase: VectorE is the bottleneck; use ScalarE
                        nc.scalar.activation(
                            so[:, nh * 512 : (nh + 1) * 512], ps[:],
                            mybir.ActivationFunctionType.Copy,
                        )
                    else:
                        nc.vector.tensor_copy(so[:, nh * 512 : (nh + 1) * 512], ps[:])
                    if tail:
                        nc.sync.dma_start(
                            out=out[tt * 128 : (tt + 1) * 128, nh * 512 : (nh + 1) * 512],
                            in_=so[:, nh * 512 : (nh + 1) * 512],
                        )
                if not tail:
                    nc.sync.dma_start(out=out[tt * 128 : (tt + 1) * 128, :], in_=so[:])

            # ---- pre-attention fillers: all 16 V tiles (the V matmuls are
            # the PE's workload while the input DMA streams in) and pair 0's
            # Q0/K0 ----
            for tt in range(16):
                v_unit(tt, "o0" if tt % 2 == 0 else "o1")
            qk_unit(0, 0, 0, "o0")
            qk_unit(0, 1, 0, "o1")

            # ---- attention, i-chunk outer; normalization and filler units
            # run at each (ci, pair) boundary ----
            seq = [(ci, pair) for ci in range(NI) for pair in range(NPAIR)]
            o_next = None
            for idx, (ci, pair) in enumerate(seq):
                njt = 4 * (ci + 1)
                if True:
                    if ci == 0 and pair < NPAIR - 1:
                        qk_unit(pair + 1, 0, 0, "o0")
                        qk_unit(pair + 1, 1, 0, "o1")
                    if ci >= 1:
                        # K chunk ci: first needed at this pair's diagonal
                        # tiles (jt >= 4*ci), so it fills this ci's
                        # ACT-bound stretch instead of crowding ci-1
                        qk_unit(pair, 1, ci, "o1")
                    if o_next is None:
                        o0 = o_ps.tile([D + 1, 512], f32, tag="o0", name=f"o0_{ci}_{pair}")
                        o1 = o_ps.tile([D + 1, 512], f32, tag="o1", name=f"o1_{ci}_{pair}")
                    else:
                        o0, o1 = o_next
                    for jt in range(njt):
                        r = jt - 4 * ci
                        i0 = 128 * r if r >= 0 else 0
                        st = ps_pool.tile([128, 2, 512], f32, tag="s", name=f"st{ci}_{pair}_{jt}")
                        for s in range(2):
                            nc.tensor.matmul(
                                st[:, s, i0:],
                                lhsT=KT_sb[
                                    64 * s : 64 * (s + 1),
                                    pair,
                                    jt * 128 : (jt + 1) * 128,
                                ],
                                rhs=QT_sb[
                                    64 * s : 64 * (s + 1),
                                    pair,
                                    ci * 512 + i0 : (ci + 1) * 512,
                                ],
                                start=True,
                                stop=True,
                            )
                        pt = p_pool.tile([128, 2, 512], bf, tag="p", name=f"pt{ci}_{pair}_{jt}")
                        nc.scalar.activation(pt[:, :, i0:], st[:, :, i0:], Exp, scale=0.125)
                        if r >= 0:
                            nc.vector.tensor_mul(
                                pt[:, :, i0:], pt[:, :, i0:], m_sb[:, r, :, i0:]
                            )
                        for s, ot in enumerate((o0, o1)):
                            nc.tensor.matmul(
                                ot[:, i0:],
                                lhsT=V_sb[:, jt, 2 * pair + s, :],
                                rhs=pt[:, s, i0:],
                                start=(jt == 0),
                                stop=(jt == njt - 1),
                            )
                    def boundary(ci=ci, pair=pair, o0=o0, o1=o1):
                        for s, ot in enumerate((o0, o1)):
                            rc = r_pool.tile([1, 512], f32, tag="rc", name=f"rc{ci}_{pair}_{s}")
                            dn = r_pool.tile([1, 512], f32, tag="dn", name=f"dn{ci}_{pair}_{s}")
                            if ci == NI - 1 and pair >= 2:
                                # drain phase: VectorE is the serial bottleneck
                                nc.scalar.activation(
                                    dn[:], ot[D : D + 1, :],
                                    mybir.ActivationFunctionType.Copy,
                                )
                            else:
                                nc.vector.tensor_copy(dn[:], ot[D : D + 1, :])
                            nc.vector.reciprocal_approx_fast(rc[:], dn[:])
                            rb = r_pool.tile([128, 512], f32, tag="rb", name=f"rb{ci}_{pair}_{s}")
                            nc.gpsimd.partition_broadcast(rb[:], rc[:])
                            nc.vector.tensor_mul(
                                AT_sb[
                                    64 * s : 64 * (s + 1),
                                    pair,
                                    ci * 512 : (ci + 1) * 512,
                                ],
                                ot[0:D, :],
                                rb[64 * s : 64 * (s + 1), :],
                            )
                    boundary()
                    # hoist the next pair's o-psum allocations ahead of the
                    # boundary fillers: keeps the accumulator rotation
                    # distance at 2 so the next pair's PV does not wait on
                    # this pair's normalization
                    if idx + 1 < len(seq):
                        nci, npair = seq[idx + 1]
                        o_next = (
                            o_ps.tile([D + 1, 512], f32, tag="o0", name=f"o0_{nci}_{npair}"),
                            o_ps.tile([D + 1, 512], f32, tag="o1", name=f"o1_{nci}_{npair}"),
                        )
                    # just-in-time fillers for upcoming chunks / proj drain
                    if ci < NI - 1:
                        qk_unit(pair, 0, ci + 1, "o0")
                    if ci >= 1 and not (ci == NI - 1 and pair >= 2):
                        proj_unit(4 * (ci - 1) + pair)
            # proj 10/11 depend only on i-chunk 2: they overlap the last
            # pair's normalization chain and keep the PE (HAM) warm
            proj_unit(10)
            proj_unit(11)
            for tt in range(12, 16):
                proj_unit(tt, tail=True)
    return nc


def _get_compiled():
    global _compiled
    if _compiled is None:
        from concourse import bacc

        nc = bacc.Bacc(
            "TRN2", target_bir_lowering=False, debug=False, num_devices=N_CORES
        )
        _build(nc)
        nc.compile()
        _compiled = nc
    return _compiled


def _shard_inputs(x, w_qkv, b_qkv, w_proj):
    """Build the 8 per-core input dicts (host-side transpose/slice/cast)."""
    in_maps = []
    wq_f, wk_f, wv_f = w_qkv[:, :C], w_qkv[:, C : 2 * C], w_qkv[:, 2 * C :]
    for c in range(N_CORES):
        b, g = c // 2, c % 2
        sl = slice(g * CL, (g + 1) * CL)
        bqs = np.ascontiguousarray(b_qkv[0 * C :][sl].reshape(NPAIR, 128).T)
        bks = np.ascontiguousarray(b_qkv[1 * C :][sl].reshape(NPAIR, 128).T)
        bvs = np.ascontiguousarray(
            np.broadcast_to(b_qkv[2 * C :][sl][None, :], (128, CL))
        )
        in_maps.append(
            {
                "xT": np.ascontiguousarray(x[b].T).astype(BF16),
                "wq": np.ascontiguousarray(wq_f[:, sl]).astype(BF16),
                "wk": np.ascontiguousarray(wk_f[:, sl]).astype(BF16),
                "wv": np.ascontiguousarray(wv_f[:, sl]).astype(BF16),
                "bq": bqs.astype(np.float32),
                "bk": bks.astype(np.float32),
                "bv": bvs.astype(np.float32),
                "wp": np.ascontiguousarray(w_proj[sl, :]).astype(BF16),
            }
        )
    return in_maps


def kernel(x, w_qkv, b_qkv, w_proj, b_proj, _trace=False, _tmpdir=None):
    from concourse.bass_utils import run_bass_kernel_spmd

    x = np.asarray(x, dtype=np.float32)
    w_qkv = np.asarray(w_qkv, dtype=np.float32)
    b_qkv = np.asarray(b_qkv, dtype=np.float32)
    w_proj = np.asarray(w_proj, dtype=np.float32)
    b_proj = np.asarray(b_proj, dtype=np.float32)

    nc = _get_compiled()
    in_maps = _shard_inputs(x, w_qkv, b_qkv, w_proj)
    res = run_bass_kernel_spmd(
        nc,
        in_maps,
        core_ids=list(range(N_CORES)),
        trace=_trace,
        tmpdir=_tmpdir,
    )
    out = np.empty((B, T, C), dtype=np.float32)
    for b in range(B):
        out[b] = (
            res.results[2 * b]["out"].astype(np.float32)
            + res.results[2 * b + 1]["out"].astype(np.float32)
            + b_proj
        )
    kernel._last_result = res
    return out


# revision 15
# speedup vs baseline: 1.0246x; 1.0139x over previous
"""Causal self-attention on 8 Trainium2 NeuronCores.

Problem (hardcoded): B=4, T=2048, C=1024, H=16, D=64.
  qkv = x @ w_qkv + b_qkv ; per-head causal softmax attention ; out = attn @ w_proj + b_proj

Sharding (per hint): tensor-parallel over heads x data-parallel over batch.
  core c -> batch b = c // 2, head group g = c % 2 (heads g*8 .. g*8+7).
Each core computes QKV for its 8 heads, causal attention, and a partial
projection (its 512 input channels of w_proj). Host sums the two partials per
batch and adds b_proj.

On-core layout ("transposed" attention so softmax reduction lands on the
matmul contraction axis):
  xT   [C, T]  (host pre-transposed, bf16)
  QT,KT [d, t] per head, 2 heads stacked per 128 partitions
  V_aug [t, 65] per head (col 64 = ones -> PV matmul emits softmax denom)
  S^T  [j, i] tiles from lhsT=KT, rhs=QT (K=64 contraction); the head pair's
       two S tiles land in one [128, 2, 512] PSUM tile (2 banks) so a single
       Exp activation serves both heads; the two matmuls are row-tiled
       (rows 0-63 / 64-127) and run concurrently on the PE.
  P = exp(S^T/8) (ScalarE, PSUM->SBUF bf16); causal diagonal tiles masked by
       a precomputed 0/1 multiply (VectorE); off-diagonal j>i tiles skipped.
  O_aug^T [65, i] accumulated over j chunks per head; row 64 = denominator.
  AT = O^T * (1/denom) broadcast -> proj lhsT; partial = A @ w_proj_slice.

Optimizations vs the 352us baseline (now ~287us):
  - diagonal narrowing: for diagonal j-tile r (= jt - 4*ci) only the
    i-range [128*r, 512) survives the causal mask; S matmul, Exp, mask
    mul and PV all restrict to that window.
  - consolidated priority-ordered DMA (each dma_start costs ~650ns of
    serial sync-engine time) with subtile deps for pipelined consumption.
  - QKV/proj production decomposed into 8-matmul units: all 16 V tiles
    up front (the PE workload while input DMA streams), Q/K chunk ci+1
    and proj chunk ci-1 emitted at the (ci, pair) boundaries; filler
    units draw PSUM from the o0/o1 tags so the "s" tag stays dedicated
    to the S->Exp double buffer.
  - drain phase: proj 10/11 (ci2-dependent) bridge the last pair's
    normalization; tail proj copies split across VectorE/ScalarE with
    per-half DMA.
"""

import numpy as np
import ml_dtypes

B, T, C, H, D = 4, 2048, 1024, 16, 64
HL = H // 2          # heads per core
CL = HL * D          # local channels (512)
NPAIR = HL // 2      # head pairs per core (4)
CCH = C // 128       # contraction chunks for qkv (8)
PCH = CL // 128      # contraction chunks for proj (4)
TT = T // 128        # t tiles (16)
NI = T // 512        # i chunks (4)
N_CORES = 8
BF16 = ml_dtypes.bfloat16

_compiled = None


def _build(nc):
    import concourse.tile as tile
    from concourse import mybir

    bf = mybir.dt.bfloat16
    f32 = mybir.dt.float32
    Exp = mybir.ActivationFunctionType.Exp

    xT = nc.dram_tensor("xT", [C, T], bf, kind="ExternalInput").ap()
    wq = nc.dram_tensor("wq", [C, CL], bf, kind="ExternalInput").ap()
    wk = nc.dram_tensor("wk", [C, CL], bf, kind="ExternalInput").ap()
    wv = nc.dram_tensor("wv", [C, CL], bf, kind="ExternalInput").ap()
    bq = nc.dram_tensor("bq", [128, NPAIR], f32, kind="ExternalInput").ap()
    bk = nc.dram_tensor("bk", [128, NPAIR], f32, kind="ExternalInput").ap()
    bv = nc.dram_tensor("bv", [128, CL], f32, kind="ExternalInput").ap()
    wp = nc.dram_tensor("wp", [CL, C], bf, kind="ExternalInput").ap()
    out = nc.dram_tensor("out", [T, C], bf, kind="ExternalOutput").ap()

    xT_r = xT.rearrange("(cc p) t -> p cc t", p=128)
    wq_r = wq.rearrange("(cc p) m -> p cc m", p=128)
    wk_r = wk.rearrange("(cc p) m -> p cc m", p=128)
    wv_r = wv.rearrange("(cc p) m -> p cc m", p=128)
    wp_r = wp.rearrange("(cc p) n -> p cc n", p=128)

    with tile.TileContext(nc) as tc:
        import contextlib

        with contextlib.ExitStack() as ctx:
            persist = ctx.enter_context(tc.tile_pool(name="persist", bufs=1))
            # PSUM budget (8 banks): "s" [128,2,512] f32 = 2 banks x 2 bufs;
            # o0/o1 [65,512] f32 = 1 bank x 2 bufs each (the two bufs hold
            # the two in-flight head-pairs' accumulators). Filler units draw
            # 1-bank tiles from the o0/o1 tags so "s" stays dedicated to the
            # S->Exp pipeline.
            ps_pool = ctx.enter_context(tc.tile_pool(name="ps_pool", bufs=2, space="PSUM"))
            o_ps = ctx.enter_context(tc.tile_pool(name="o_ps", bufs=2, space="PSUM"))
            p_pool = ctx.enter_context(tc.tile_pool(name="p_pool", bufs=6))
            r_pool = ctx.enter_context(tc.tile_pool(name="r_pool", bufs=4))
            st_pool = ctx.enter_context(tc.tile_pool(name="st_pool", bufs=3))

            # ---- persistent SBUF tensors ----
            xT_sb = persist.tile([128, CCH, T], bf)
            wq_sb = persist.tile([128, CCH, CL], bf)
            wk_sb = persist.tile([128, CCH, CL], bf)
            wv_sb = persist.tile([128, CCH, CL], bf)
            wp_sb = persist.tile([128, PCH, C], bf)
            bq_sb = persist.tile([128, NPAIR], f32)
            bk_sb = persist.tile([128, NPAIR], f32)
            bv_sb = persist.tile([128, CL], f32)
            QT_sb = persist.tile([128, NPAIR, T], bf)
            KT_sb = persist.tile([128, NPAIR, T], bf)
            V_sb = persist.tile([128, TT, HL, D + 1], bf)
            AT_sb = persist.tile([128, PCH, T], bf)
            m_sb = persist.tile([128, 4, 2, 512], bf)

            # ---- DMA: few large transfers, priority order. The sync engine
            # issues each dma_start serially, so count matters; subtile deps
            # let consumers start as soon as their transfer lands. ----
            TH = T // 2
            nc.sync.dma_start(out=wv_sb[:, 0:2, :], in_=wv_r[:, 0:2, :])
            nc.sync.dma_start(out=xT_sb[:, 0:2, :TH], in_=xT_r[:, 0:2, :TH])
            nc.sync.dma_start(out=wv_sb[:, 2:8, :], in_=wv_r[:, 2:8, :])
            nc.sync.dma_start(out=xT_sb[:, 2:4, :TH], in_=xT_r[:, 2:4, :TH])
            nc.sync.dma_start(out=xT_sb[:, 4:8, :TH], in_=xT_r[:, 4:8, :TH])
            nc.sync.dma_start(out=bv_sb[:], in_=bv[:])
            nc.sync.dma_start(out=xT_sb[:, 0:4, TH:], in_=xT_r[:, 0:4, TH:])
            nc.sync.dma_start(out=xT_sb[:, 4:8, TH:], in_=xT_r[:, 4:8, TH:])
            nc.sync.dma_start(out=wq_sb[:, 0:4, :], in_=wq_r[:, 0:4, :])
            nc.sync.dma_start(out=wq_sb[:, 4:8, :], in_=wq_r[:, 4:8, :])
            nc.sync.dma_start(out=wk_sb[:, 0:4, :], in_=wk_r[:, 0:4, :])
            nc.sync.dma_start(out=wk_sb[:, 4:8, :], in_=wk_r[:, 4:8, :])
            nc.sync.dma_start(out=bq_sb[:], in_=bq[:])
            nc.sync.dma_start(out=bk_sb[:], in_=bk[:])
            nc.sync.dma_start(out=wp_sb[:, 0:2, :], in_=wp_r[:, 0:2, :])
            nc.sync.dma_start(out=wp_sb[:, 2:4, :], in_=wp_r[:, 2:4, :])

            # causal 0/1 masks, replicated for the pair dim:
            # m[r][jj, :, ii] = 1 if ii - jj >= 128*r else 0
            for r in range(4):
                nc.vector.memset(m_sb[:, r], 1.0)
                nc.gpsimd.affine_select(
                    out=m_sb[:, r],
                    in_=m_sb[:, r],
                    compare_op=mybir.AluOpType.is_ge,
                    fill=0.0,
                    base=-128 * r,
                    pattern=[[0, 2], [1, 512]],
                    channel_multiplier=-1,
                )
            # ones column of V_aug
            nc.vector.memset(V_sb[:, :, :, D], 1.0)

            # ---- filler units: 8-matmul groups, PSUM from o0/o1 tags ----
            def v_unit(tt, ftag):
                ps = o_ps.tile([128, 512], f32, tag=ftag, name=f"vps{tt}")
                for cc in range(CCH):
                    nc.tensor.matmul(
                        ps[:],
                        lhsT=xT_sb[:, cc, tt * 128 : (tt + 1) * 128],
                        rhs=wv_sb[:, cc, :],
                        start=(cc == 0),
                        stop=(cc == CCH - 1),
                    )
                nc.vector.tensor_add(
                    V_sb[:, tt, :, 0:D],
                    ps.rearrange("p (h d) -> p h d", h=HL),
                    bv_sb[:].rearrange("p (h d) -> p h d", h=HL),
                )

            def qk_unit(pair, which, tc_, ftag):
                w_sb, dst, b_sb = (
                    (wq_sb, QT_sb, bq_sb) if which == 0 else (wk_sb, KT_sb, bk_sb)
                )
                ps = o_ps.tile([128, 512], f32, tag=ftag, name=f"qkps{pair}_{which}_{tc_}")
                t0 = tc_ * 512
                for cc in range(CCH):
                    nc.tensor.matmul(
                        ps[:],
                        lhsT=w_sb[:, cc, pair * 128 : (pair + 1) * 128],
                        rhs=xT_sb[:, cc, t0 : t0 + 512],
                        start=(cc == 0),
                        stop=(cc == CCH - 1),
                    )
                nc.vector.tensor_scalar_add(
                    dst[:, pair, t0 : t0 + 512], ps[:], b_sb[:, pair : pair + 1]
                )

            def proj_unit(tt, tail=False):
                so = st_pool.tile([128, C], bf, tag="so", name=f"so{tt}")
                for nh in range(2):
                    ps = o_ps.tile(
                        [128, 512], f32, tag=("o0" if nh == 0 else "o1"), name=f"pps{tt}_{nh}"
                    )
                    for cc in range(PCH):
                        nc.tensor.matmul(
                            ps[:],
                            lhsT=AT_sb[:, cc, tt * 128 : (tt + 1) * 128],
                            rhs=wp_sb[:, cc, nh * 512 : (nh + 1) * 512],
                            start=(cc == 0),
                            stop=(cc == PCH - 1),
                        )
                    if tail and nh == 1:
                        # drain phase: VectorE is the bottleneck; use ScalarE
                        nc.scalar.activation(
                            so[:, nh * 512 : (nh + 1) * 512], ps[:],
                            mybir.ActivationFunctionType.Copy,
                        )
                    else:
                        nc.vector.tensor_copy(so[:, nh * 512 : (nh + 1) * 512], ps[:])
                    if tail:
                        nc.sync.dma_start(
                            out=out[tt * 128 : (tt + 1) * 128, nh * 512 : (nh + 1) * 512],
                            in_=so[:, nh * 512 : (nh + 1) * 512],
                        )
                if not tail:
                    nc.sync.dma_start(out=out[tt * 128 : (tt + 1) * 128, :], in_=so[:])

            # ---- pre-attention fillers: all 16 V tiles (the V matmuls are
            # the PE's workload while the input DMA streams in) and pair 0's
            # Q0/K0 ----
            for tt in range(16):
                v_unit(tt, "o0" if tt % 2 == 0 else "o1")
            qk_unit(0, 0, 0, "o0")
            qk_unit(0, 1, 0, "o1")

            # ---- attention, i-chunk outer; normalization and filler units
            # run at each (ci, pair) boundary ----
            seq = [(ci, pair) for ci in range(NI) for pair in range(NPAIR)]
            o_next = None
            for idx, (ci, pair) in enumerate(seq):
                njt = 4 * (ci + 1)
                if True:
                    if ci == 0 and pair < NPAIR - 1:
                        qk_unit(pair + 1, 0, 0, "o0")
                        qk_unit(pair + 1, 1, 0, "o1")
                    if ci >= 1:
                        # K chunk ci: first needed at this pair's diagonal
                        # tiles (jt >= 4*ci), so it fills this ci's
                        # ACT-bound stretch instead of crowding ci-1
                        qk_unit(pair, 1, ci, "o1")
                    if o_next is None:
                        o0 = o_ps.tile([D + 1, 512], f32, tag="o0", name=f"o0_{ci}_{pair}")
                        o1 = o_ps.tile([D + 1, 512], f32, tag="o1", name=f"o1_{ci}_{pair}")
                    else:
                        o0, o1 = o_next
                    for jt in range(njt):
                        r = jt - 4 * ci
                        i0 = 128 * r if r >= 0 else 0
                        st = ps_pool.tile([128, 2, 512], f32, tag="s", name=f"st{ci}_{pair}_{jt}")
                        for s in range(2):
                            nc.tensor.matmul(
                                st[:, s, i0:],
                                lhsT=KT_sb[
                                    64 * s : 64 * (s + 1),
                                    pair,
                                    jt * 128 : (jt + 1) * 128,
                                ],
                                rhs=QT_sb[
                                    64 * s : 64 * (s + 1),
                                    pair,
                                    ci * 512 + i0 : (ci + 1) * 512,
                                ],
                                start=True,
                                stop=True,
                            )
                        pt = p_pool.tile([128, 2, 512], bf, tag="p", name=f"pt{ci}_{pair}_{jt}")
                        nc.scalar.activation(pt[:, :, i0:], st[:, :, i0:], Exp, scale=0.125)
                        if r >= 0:
                            nc.vector.tensor_mul(
                                pt[:, :, i0:], pt[:, :, i0:], m_sb[:, r, :, i0:]
                            )
                        for s, ot in enumerate((o0, o1)):
                            nc.tensor.matmul(
                                ot[:, i0:],
                                lhsT=V_sb[:, jt, 2 * pair + s, :],
                                rhs=pt[:, s, i0:],
                                start=(jt == 0),
                                stop=(jt == njt - 1),
                            )
                    def boundary(ci=ci, pair=pair, o0=o0, o1=o1):
                        for s, ot in enumerate((o0, o1)):
                            rc = r_pool.tile([1, 512], f32, tag="rc", name=f"rc{ci}_{pair}_{s}")
                            dn = r_pool.tile([1, 512], f32, tag="dn", name=f"dn{ci}_{pair}_{s}")
                            if ci == NI - 1 and pair >= 2:
                                # drain phase: VectorE is the serial bottleneck
                                nc.scalar.activation(
                                    dn[:], ot[D : D + 1, :],
                                    mybir.ActivationFunctionType.Copy,
                                )
                            else:
                                nc.vector.tensor_copy(dn[:], ot[D : D + 1, :])
                            nc.vector.reciprocal_approx_fast(rc[:], dn[:])
                            rb = r_pool.tile([128, 512], f32, tag="rb", name=f"rb{ci}_{pair}_{s}")
                            nc.gpsimd.partition_broadcast(rb[:], rc[:])
                            nc.vector.tensor_mul(
                                AT_sb[
                                    64 * s : 64 * (s + 1),
                                    pair,
                                    ci * 512 : (ci + 1) * 512,
                                ],
                                ot[0:D, :],
                                rb[64 * s : 64 * (s + 1), :],
                            )
                    boundary()
                    # hoist the next pair's o-psum allocations ahead of the
                    # boundary fillers: keeps the accumulator rotation
                    # distance at 2 so the next pair's PV does not wait on
                    # this pair's normalization
                    if idx + 1 < len(seq):
                        nci, npair = seq[idx + 1]
                        o_next = (
                            o_ps.tile([D + 1, 512], f32, tag="o0", name=f"o0_{nci}_{npair}"),
                            o_ps.tile([D + 1, 512], f32, tag="o1", name=f"o1_{nci}_{npair}"),
                        )
                    # just-in-time fillers for upcoming chunks / proj drain
                    if ci < NI - 1:
                        qk_unit(pair, 0, ci + 1, "o0")
                    if ci >= 1 and not (ci == NI - 1 and pair >= 2):
                        proj_unit(4 * (ci - 1) + pair)
            # proj 10/11 depend only on i-chunk 2: they overlap the last
            # pair's normalization chain and keep the PE (HAM) warm
            proj_unit(10)
            proj_unit(11)
            for tt in range(12, 16):
                proj_unit(tt, tail=True)
    return nc


def _get_compiled():
    global _compiled
    if _compiled is None:
        from concourse import bacc

        nc = bacc.Bacc(
            "TRN2", target_bir_lowering=False, debug=False, num_devices=N_CORES
        )
        _build(nc)
        nc.compile()
        _compiled = nc
    return _compiled


def _shard_inputs(x, w_qkv, b_qkv, w_proj):
    """Build the 8 per-core input dicts (host-side transpose/slice/cast)."""
    in_maps = []
    wq_f, wk_f, wv_f = w_qkv[:, :C], w_qkv[:, C : 2 * C], w_qkv[:, 2 * C :]
    for c in range(N_CORES):
        b, g = c // 2, c % 2
        sl = slice(g * CL, (g + 1) * CL)
        bqs = np.ascontiguousarray(b_qkv[0 * C :][sl].reshape(NPAIR, 128).T)
        bks = np.ascontiguousarray(b_qkv[1 * C :][sl].reshape(NPAIR, 128).T)
        bvs = np.ascontiguousarray(
            np.broadcast_to(b_qkv[2 * C :][sl][None, :], (128, CL))
        )
        in_maps.append(
            {
                "xT": np.ascontiguousarray(x[b].T).astype(BF16),
                "wq": np.ascontiguousarray(wq_f[:, sl]).astype(BF16),
                "wk": np.ascontiguousarray(wk_f[:, sl]).astype(BF16),
                "wv": np.ascontiguousarray(wv_f[:, sl]).astype(BF16),
                "bq": bqs.astype(np.float32),
                "bk": bks.astype(np.float32),
                "bv": bvs.astype(np.float32),
                "wp": np.ascontiguousarray(w_proj[sl, :]).astype(BF16),
            }
        )
    return in_maps


def kernel(x, w_qkv, b_qkv, w_proj, b_proj, _trace=False, _tmpdir=None):
    from concourse.bass_utils import run_bass_kernel_spmd

    x = np.asarray(x, dtype=np.float32)
    w_qkv = np.asarray(w_qkv, dtype=np.float32)
    b_qkv = np.asarray(b_qkv, dtype=np.float32)
    w_proj = np.asarray(w_proj, dtype=np.float32)
    b_proj = np.asarray(b_proj, dtype=np.float32)

    nc = _get_compiled()
    in_maps = _shard_inputs(x, w_qkv, b_qkv, w_proj)
    res = run_bass_kernel_spmd(
        nc,
        in_maps,
        core_ids=list(range(N_CORES)),
        trace=_trace,
        tmpdir=_tmpdir,
    )
    out = np.empty((B, T, C), dtype=np.float32)
    for b in range(B):
        out[b] = (
            res.results[2 * b]["out"].astype(np.float32)
            + res.results[2 * b + 1]["out"].astype(np.float32)
            + b_proj
        )
    kernel._last_result = res
    return out


# revision 20
# speedup vs baseline: 1.0345x; 1.0097x over previous
"""Causal self-attention on 8 Trainium2 NeuronCores.

Problem (hardcoded): B=4, T=2048, C=1024, H=16, D=64.
  qkv = x @ w_qkv + b_qkv ; per-head causal softmax attention ; out = attn @ w_proj + b_proj

Sharding (per hint): tensor-parallel over heads x data-parallel over batch.
  core c -> batch b = c // 2, head group g = c % 2 (heads g*8 .. g*8+7).
Each core computes QKV for its 8 heads, causal attention, and a partial
projection (its 512 input channels of w_proj). Host sums the two partials per
batch and adds b_proj.

On-core layout ("transposed" attention so softmax reduction lands on the
matmul contraction axis):
  xT   [C, T]  (host pre-transposed, bf16)
  QT,KT [d, t] per head, 2 heads stacked per 128 partitions
  V_aug [t, 65] per head (col 64 = ones -> PV matmul emits softmax denom)
  S^T  [j, i] tiles from lhsT=KT, rhs=QT (K=64 contraction); the head pair's
       two S tiles land in one [128, 2, 512] PSUM tile (2 banks) so a single
       Exp activation serves both heads; the two matmuls are row-tiled
       (rows 0-63 / 64-127) and run concurrently on the PE.
  P = exp(S^T/8) (ScalarE, PSUM->SBUF bf16); causal diagonal tiles masked by
       a precomputed 0/1 multiply (VectorE); off-diagonal j>i tiles skipped.
  O_aug^T [65, i] accumulated over j chunks per head; row 64 = denominator.
  AT = O^T * (1/denom) broadcast -> proj lhsT; partial = A @ w_proj_slice.

Optimizations vs the 352us baseline (now ~287us):
  - diagonal narrowing: for diagonal j-tile r (= jt - 4*ci) only the
    i-range [128*r, 512) survives the causal mask; S matmul, Exp, mask
    mul and PV all restrict to that window.
  - consolidated priority-ordered DMA (each dma_start costs ~650ns of
    serial sync-engine time) with subtile deps for pipelined consumption.
  - QKV/proj production decomposed into 8-matmul units: all 16 V tiles
    up front (the PE workload while input DMA streams), Q/K chunk ci+1
    and proj chunk ci-1 emitted at the (ci, pair) boundaries; filler
    units draw PSUM from the o0/o1 tags so the "s" tag stays dedicated
    to the S->Exp double buffer.
  - drain phase: proj 10/11 (ci2-dependent) bridge the last pair's
    normalization; tail proj copies split across VectorE/ScalarE with
    per-half DMA.
"""

import numpy as np
import ml_dtypes

B, T, C, H, D = 4, 2048, 1024, 16, 64
HL = H // 2          # heads per core
CL = HL * D          # local channels (512)
NPAIR = HL // 2      # head pairs per core (4)
CCH = C // 128       # contraction chunks for qkv (8)
PCH = CL // 128      # contraction chunks for proj (4)
TT = T // 128        # t tiles (16)
NI = T // 512        # i chunks (4)
N_CORES = 8
BF16 = ml_dtypes.bfloat16

_compiled = None


def _build(nc):
    import concourse.tile as tile
    from concourse import mybir

    bf = mybir.dt.bfloat16
    f32 = mybir.dt.float32
    Exp = mybir.ActivationFunctionType.Exp

    xT = nc.dram_tensor("xT", [C, T], bf, kind="ExternalInput").ap()
    wq = nc.dram_tensor("wq", [C, CL], bf, kind="ExternalInput").ap()
    wk = nc.dram_tensor("wk", [C, CL], bf, kind="ExternalInput").ap()
    wv = nc.dram_tensor("wv", [C, CL], bf, kind="ExternalInput").ap()
    bq = nc.dram_tensor("bq", [128, NPAIR], f32, kind="ExternalInput").ap()
    bk = nc.dram_tensor("bk", [128, NPAIR], f32, kind="ExternalInput").ap()
    bv = nc.dram_tensor("bv", [128, CL], f32, kind="ExternalInput").ap()
    wp = nc.dram_tensor("wp", [CL, C], bf, kind="ExternalInput").ap()
    out = nc.dram_tensor("out", [T, C], bf, kind="ExternalOutput").ap()

    xT_r = xT.rearrange("(cc p) t -> p cc t", p=128)
    wq_r = wq.rearrange("(cc p) m -> p cc m", p=128)
    wk_r = wk.rearrange("(cc p) m -> p cc m", p=128)
    wv_r = wv.rearrange("(cc p) m -> p cc m", p=128)
    wp_r = wp.rearrange("(cc p) n -> p cc n", p=128)

    with tile.TileContext(nc) as tc:
        import contextlib

        with contextlib.ExitStack() as ctx:
            persist = ctx.enter_context(tc.tile_pool(name="persist", bufs=1))
            # PSUM budget (8 banks): "s" [128,2,512] f32 = 2 banks x 2 bufs;
            # o0/o1 [65,512] f32 = 1 bank x 2 bufs each (the two bufs hold
            # the two in-flight head-pairs' accumulators). Filler units draw
            # 1-bank tiles from the o0/o1 tags so "s" stays dedicated to the
            # S->Exp pipeline.
            ps_pool = ctx.enter_context(tc.tile_pool(name="ps_pool", bufs=2, space="PSUM"))
            o_ps = ctx.enter_context(tc.tile_pool(name="o_ps", bufs=2, space="PSUM"))
            p_pool = ctx.enter_context(tc.tile_pool(name="p_pool", bufs=6))
            r_pool = ctx.enter_context(tc.tile_pool(name="r_pool", bufs=4))
            st_pool = ctx.enter_context(tc.tile_pool(name="st_pool", bufs=3))

            # ---- persistent SBUF tensors ----
            xT_sb = persist.tile([128, CCH, T], bf)
            wq_sb = persist.tile([128, CCH, CL], bf)
            wk_sb = persist.tile([128, CCH, CL], bf)
            wv_sb = persist.tile([128, CCH, CL], bf)
            wp_sb = persist.tile([128, PCH, C], bf)
            bq_sb = persist.tile([128, NPAIR], f32)
            bk_sb = persist.tile([128, NPAIR], f32)
            bv_sb = persist.tile([128, CL], f32)
            QT_sb = persist.tile([128, NPAIR, T], bf)
            KT_sb = persist.tile([128, NPAIR, T], bf)
            V_sb = persist.tile([128, TT, HL, D + 1], bf)
            AT_sb = persist.tile([128, PCH, T], bf)
            m_sb = persist.tile([128, 4, 2, 512], bf)

            # ---- DMA: few large transfers, priority order. The sync engine
            # issues each dma_start serially, so count matters; subtile deps
            # let consumers start as soon as their transfer lands. ----
            TH = T // 2
            nc.sync.dma_start(out=wv_sb[:, 0:2, :], in_=wv_r[:, 0:2, :])
            nc.sync.dma_start(out=xT_sb[:, 0:2, :TH], in_=xT_r[:, 0:2, :TH])
            nc.sync.dma_start(out=wv_sb[:, 2:8, :], in_=wv_r[:, 2:8, :])
            nc.sync.dma_start(out=xT_sb[:, 2:4, :TH], in_=xT_r[:, 2:4, :TH])
            nc.sync.dma_start(out=xT_sb[:, 4:8, :TH], in_=xT_r[:, 4:8, :TH])
            nc.sync.dma_start(out=bv_sb[:], in_=bv[:])
            # bq/bk are 2KB but the queue delivers in order: issue them
            # before the 2MB wq/wk streams so the first qk bias-add (and
            # with it the first S matmul) is not held ~3us behind wk
            nc.sync.dma_start(out=bq_sb[:], in_=bq[:])
            nc.sync.dma_start(out=bk_sb[:], in_=bk[:])
            nc.sync.dma_start(out=xT_sb[:, 0:4, TH:], in_=xT_r[:, 0:4, TH:])
            nc.sync.dma_start(out=xT_sb[:, 4:8, TH:], in_=xT_r[:, 4:8, TH:])
            nc.sync.dma_start(out=wq_sb[:, 0:4, :], in_=wq_r[:, 0:4, :])
            nc.sync.dma_start(out=wq_sb[:, 4:8, :], in_=wq_r[:, 4:8, :])
            nc.sync.dma_start(out=wk_sb[:, 0:4, :], in_=wk_r[:, 0:4, :])
            nc.sync.dma_start(out=wk_sb[:, 4:8, :], in_=wk_r[:, 4:8, :])
            nc.sync.dma_start(out=wp_sb[:, 0:2, :], in_=wp_r[:, 0:2, :])
            nc.sync.dma_start(out=wp_sb[:, 2:4, :], in_=wp_r[:, 2:4, :])

            # causal 0/1 masks, replicated for the pair dim:
            # m[r][jj, :, ii] = 1 if ii - jj >= 128*r else 0
            for r in range(4):
                nc.vector.memset(m_sb[:, r], 1.0)
                nc.gpsimd.affine_select(
                    out=m_sb[:, r],
                    in_=m_sb[:, r],
                    compare_op=mybir.AluOpType.is_ge,
                    fill=0.0,
                    base=-128 * r,
                    pattern=[[0, 2], [1, 512]],
                    channel_multiplier=-1,
                )
            # ones column of V_aug
            nc.vector.memset(V_sb[:, :, :, D], 1.0)

            # ---- filler units: 8-matmul groups, PSUM from o0/o1 tags ----
            def v_unit(tt, ftag):
                ps = o_ps.tile([128, 512], f32, tag=ftag, name=f"vps{tt}")
                for cc in range(CCH):
                    nc.tensor.matmul(
                        ps[:],
                        lhsT=xT_sb[:, cc, tt * 128 : (tt + 1) * 128],
                        rhs=wv_sb[:, cc, :],
                        start=(cc == 0),
                        stop=(cc == CCH - 1),
                    )
                nc.vector.tensor_add(
                    V_sb[:, tt, :, 0:D],
                    ps.rearrange("p (h d) -> p h d", h=HL),
                    bv_sb[:].rearrange("p (h d) -> p h d", h=HL),
                )

            def qk_unit(pair, which, tc_, ftag):
                w_sb, dst, b_sb = (
                    (wq_sb, QT_sb, bq_sb) if which == 0 else (wk_sb, KT_sb, bk_sb)
                )
                ps = o_ps.tile([128, 512], f32, tag=ftag, name=f"qkps{pair}_{which}_{tc_}")
                t0 = tc_ * 512
                for cc in range(CCH):
                    nc.tensor.matmul(
                        ps[:],
                        lhsT=w_sb[:, cc, pair * 128 : (pair + 1) * 128],
                        rhs=xT_sb[:, cc, t0 : t0 + 512],
                        start=(cc == 0),
                        stop=(cc == CCH - 1),
                    )
                nc.vector.tensor_scalar_add(
                    dst[:, pair, t0 : t0 + 512], ps[:], b_sb[:, pair : pair + 1]
                )

            def proj_unit(tt, tail=False):
                so = st_pool.tile([128, C], bf, tag="so", name=f"so{tt}")
                for nh in range(2):
                    ps = o_ps.tile(
                        [128, 512], f32, tag=("o0" if nh == 0 else "o1"), name=f"pps{tt}_{nh}"
                    )
                    for cc in range(PCH):
                        nc.tensor.matmul(
                            ps[:],
                            lhsT=AT_sb[:, cc, tt * 128 : (tt + 1) * 128],
                            rhs=wp_sb[:, cc, nh * 512 : (nh + 1) * 512],
                            start=(cc == 0),
                            stop=(cc == PCH - 1),
                        )
                    if tail and nh == 1:
                        # drain phase: VectorE is the bottleneck; use ScalarE
                        nc.scalar.activation(
                            so[:, nh * 512 : (nh + 1) * 512], ps[:],
                            mybir.ActivationFunctionType.Copy,
                        )
                    else:
                        nc.vector.tensor_copy(so[:, nh * 512 : (nh + 1) * 512], ps[:])
                    if tail:
                        nc.sync.dma_start(
                            out=out[tt * 128 : (tt + 1) * 128, nh * 512 : (nh + 1) * 512],
                            in_=so[:, nh * 512 : (nh + 1) * 512],
                        )
                if not tail:
                    nc.sync.dma_start(out=out[tt * 128 : (tt + 1) * 128, :], in_=so[:])

            # ---- pre-attention fillers: all 16 V tiles (the V matmuls are
            # the PE's workload while the input DMA streams in) and pair 0's
            # Q0/K0 ----
            for tt in range(16):
                v_unit(tt, "o0" if tt % 2 == 0 else "o1")
            qk_unit(0, 0, 0, "o0")
            qk_unit(0, 1, 0, "o1")

            # ---- attention, i-chunk outer; normalization and filler units
            # run at each (ci, pair) boundary ----
            seq = [(ci, pair) for ci in range(NI) for pair in range(NPAIR)]
            o_next = None
            for idx, (ci, pair) in enumerate(seq):
                njt = 4 * (ci + 1)
                if True:
                    if ci == 0 and pair < NPAIR - 1:
                        qk_unit(pair + 1, 0, 0, "o0")
                        qk_unit(pair + 1, 1, 0, "o1")
                    if ci >= 1:
                        # K chunk ci: first needed at this pair's diagonal
                        # tiles (jt >= 4*ci), so it fills this ci's
                        # ACT-bound stretch instead of crowding ci-1
                        qk_unit(pair, 1, ci, "o1")
                    if o_next is None:
                        o0 = o_ps.tile([D + 1, 512], f32, tag="o0", name=f"o0_{ci}_{pair}")
                        o1 = o_ps.tile([D + 1, 512], f32, tag="o1", name=f"o1_{ci}_{pair}")
                    else:
                        o0, o1 = o_next
                    for jt in range(njt):
                        r = jt - 4 * ci
                        i0 = 128 * r if r >= 0 else 0
                        st = ps_pool.tile([128, 2, 512], f32, tag="s", name=f"st{ci}_{pair}_{jt}")
                        for s in range(2):
                            nc.tensor.matmul(
                                st[:, s, i0:],
                                lhsT=KT_sb[
                                    64 * s : 64 * (s + 1),
                                    pair,
                                    jt * 128 : (jt + 1) * 128,
                                ],
                                rhs=QT_sb[
                                    64 * s : 64 * (s + 1),
                                    pair,
                                    ci * 512 + i0 : (ci + 1) * 512,
                                ],
                                start=True,
                                stop=True,
                            )
                        pt = p_pool.tile([128, 2, 512], bf, tag="p", name=f"pt{ci}_{pair}_{jt}")
                        nc.scalar.activation(pt[:, :, i0:], st[:, :, i0:], Exp, scale=0.125)
                        if r >= 0:
                            nc.vector.tensor_mul(
                                pt[:, :, i0:], pt[:, :, i0:], m_sb[:, r, :, i0:]
                            )
                        for s, ot in enumerate((o0, o1)):
                            nc.tensor.matmul(
                                ot[:, i0:],
                                lhsT=V_sb[:, jt, 2 * pair + s, :],
                                rhs=pt[:, s, i0:],
                                start=(jt == 0),
                                stop=(jt == njt - 1),
                            )
                    def boundary(ci=ci, pair=pair, o0=o0, o1=o1):
                        for s, ot in enumerate((o0, o1)):
                            rc = r_pool.tile([1, 512], f32, tag="rc", name=f"rc{ci}_{pair}_{s}")
                            dn = r_pool.tile([1, 512], f32, tag="dn", name=f"dn{ci}_{pair}_{s}")
                            if ci == NI - 1 and pair >= 2:
                                # drain phase: VectorE is the serial bottleneck
                                nc.scalar.activation(
                                    dn[:], ot[D : D + 1, :],
                                    mybir.ActivationFunctionType.Copy,
                                )
                            else:
                                nc.vector.tensor_copy(dn[:], ot[D : D + 1, :])
                            nc.vector.reciprocal_approx_fast(rc[:], dn[:])
                            rb = r_pool.tile([128, 512], f32, tag="rb", name=f"rb{ci}_{pair}_{s}")
                            nc.gpsimd.partition_broadcast(rb[:], rc[:])
                            nc.vector.tensor_mul(
                                AT_sb[
                                    64 * s : 64 * (s + 1),
                                    pair,
                                    ci * 512 : (ci + 1) * 512,
                                ],
                                ot[0:D, :],
                                rb[64 * s : 64 * (s + 1), :],
                            )
                    boundary()
                    # hoist the next pair's o-psum allocations ahead of the
                    # boundary fillers: keeps the accumulator rotation
                    # distance at 2 so the next pair's PV does not wait on
                    # this pair's normalization
                    if idx + 1 < len(seq):
                        nci, npair = seq[idx + 1]
                        o_next = (
                            o_ps.tile([D + 1, 512], f32, tag="o0", name=f"o0_{nci}_{npair}"),
                            o_ps.tile([D + 1, 512], f32, tag="o1", name=f"o1_{nci}_{npair}"),
                        )
                    # just-in-time fillers for upcoming chunks / proj drain
                    if ci < NI - 1:
                        qk_unit(pair, 0, ci + 1, "o0")
                    if ci >= 1 and not (ci == NI - 1 and pair >= 2):
                        proj_unit(4 * (ci - 1) + pair)
            # proj 10/11 depend only on i-chunk 2: they overlap the last
            # pair's normalization chain and keep the PE (HAM) warm
            proj_unit(10)
            proj_unit(11)
            for tt in range(12, 16):
                proj_unit(tt, tail=True)
    return nc


def _get_compiled():
    global _compiled
    if _compiled is None:
        from concourse import bacc

        nc = bacc.Bacc(
            "TRN2", target_bir_lowering=False, debug=False, num_devices=N_CORES
        )
        _build(nc)
        nc.compile()
        _compiled = nc
    return _compiled


def _shard_inputs(x, w_qkv, b_qkv, w_proj):
    """Build the 8 per-core input dicts (host-side transpose/slice/cast)."""
    in_maps = []
    wq_f, wk_f, wv_f = w_qkv[:, :C], w_qkv[:, C : 2 * C], w_qkv[:, 2 * C :]
    for c in range(N_CORES):
        b, g = c // 2, c % 2
        sl = slice(g * CL, (g + 1) * CL)
        bqs = np.ascontiguousarray(b_qkv[0 * C :][sl].reshape(NPAIR, 128).T)
        bks = np.ascontiguousarray(b_qkv[1 * C :][sl].reshape(NPAIR, 128).T)
        bvs = np.ascontiguousarray(
            np.broadcast_to(b_qkv[2 * C :][sl][None, :], (128, CL))
        )
        in_maps.append(
            {
                "xT": np.ascontiguousarray(x[b].T).astype(BF16),
                "wq": np.ascontiguousarray(wq_f[:, sl]).astype(BF16),
                "wk": np.ascontiguousarray(wk_f[:, sl]).astype(BF16),
                "wv": np.ascontiguousarray(wv_f[:, sl]).astype(BF16),
                "bq": bqs.astype(np.float32),
                "bk": bks.astype(np.float32),
                "bv": bvs.astype(np.float32),
                "wp": np.ascontiguousarray(w_proj[sl, :]).astype(BF16),
            }
        )
    return in_maps


def kernel(x, w_qkv, b_qkv, w_proj, b_proj, _trace=False, _tmpdir=None):
    from concourse.bass_utils import run_bass_kernel_spmd

    x = np.asarray(x, dtype=np.float32)
    w_qkv = np.asarray(w_qkv, dtype=np.float32)
    b_qkv = np.asarray(b_qkv, dtype=np.float32)
    w_proj = np.asarray(w_proj, dtype=np.float32)
    b_proj = np.asarray(b_proj, dtype=np.float32)

    nc = _get_compiled()
    in_maps = _shard_inputs(x, w_qkv, b_qkv, w_proj)
    res = run_bass_kernel_spmd(
        nc,
        in_maps,
        core_ids=list(range(N_CORES)),
        trace=_trace,
        tmpdir=_tmpdir,
    )
    out = np.empty((B, T, C), dtype=np.float32)
    for b in range(B):
        out[b] = (
            res.results[2 * b]["out"].astype(np.float32)
            + res.results[2 * b + 1]["out"].astype(np.float32)
            + b_proj
        )
    kernel._last_result = res
    return out
